# revision 5
# baseline (speedup 1.0000x reference)
"""Trainium2 Bass kernel for nn_AttentionBlock (GroupNorm + windowed MHA + proj + residual).

Contract: kernel(**inputs) takes FULL unsharded inputs (as from reference.setup_inputs())
and returns the FULL output [1, 256, 96, 96] float32.

Sharding: sequence-parallel over query positions across 8 cores. Each core gets a
uniform slice of each of the 3 reference attention windows:
  W0: q[512i   : 512(i+1)]    attends kv[0    : 6144]
  W1: q[4096+512i : ...]      attends kv[2048 : 9216]
  W2: q[8192+128i : ...]      attends kv[6144 : 9216]
All 4 heads for those queries are computed locally, so the output projection and
residual are local too. Every core redundantly computes GroupNorm stats and the
full-sequence K/V (needed since every core's windows span nearly the full sequence).

On-chip dataflow ("transposed" layout, channels on partitions):
  x [256, 9216]  -> bn_stats -> group stats via small PE matmuls -> per-channel a,b
  xn = a*x + b   (DVE, streamed)
  kT [2heads*64, 9216] = Wk-slice.T-matmuls (per pass of 2 heads)
  v  [9216, 2heads*65] (65th col = ones, so PV matmul also produces softmax sums)
  q  [2heads*64, 1152]
  S^T[keys, q] = kT-chunk as lhsT, qT as rhs; exp on ScalarE (PSUM->SBUF, batched
  SCHUNK key-chunks per ACTIVATE); PV: lhsT=[v|1], rhs=exp(S^T) accumulating O^T[65, q];
  normalize by broadcast reciprocal row (DRAM-bounce partition broadcast);
  proj: projT-matmuls over assembled attn^T [256, 1152] + bias (via rank-1 matmul)
  + residual, DMA out.

All matmul operands use float32r (TF32-like PE fast mode, ~4x fp32 throughput;
end-to-end rel err ~2e-5). Emission is software-pipelined: pass-1 qkv tiles are
emitted between pass-0 attention S-tile groups, gated so a k/v sub-tile is only
rewritten after every already-emitted reader (Tile orders by emission).
"""

import numpy as np

import concourse.bass as bass
import concourse.tile as tile
from concourse import mybir
from concourse.vector_clock import ScopedClock, VectorClock

F32 = mybir.dt.float32
F32R = mybir.dt.float32r  # fp32 storage, TF32-like PE mode: ~4x matmul throughput, rel err ~1.5e-4
AF = mybir.ActivationFunctionType
ALU = mybir.AluOpType

C = 256
SEQ = 9216
NCORES = 8
HEADS = 4
D = 64
EPS = 1e-5
SCALE = 0.125  # 1/sqrt(64)
NQC = 1152  # queries per core
ST = 512  # seq tile for qkv streaming
NST = SEQ // ST  # 18
# windows: (q_off, q_len, key_chunk0, n_key_chunks)
WINDOWS = [(0, 512, 0, 48), (512, 512, 16, 56), (1024, 128, 48, 24)]
SCHUNK = 2  # key-chunks batched per exp ACTIVATE (2 PSUM banks)


def _patch_tile_drain():
    """This container's walrus rejects >1 sem wait on one sync CTRL instruction
    ("Too many sync wait commands"). Split the TileContext-exit drain's waits
    into one-wait-per-nop instructions."""
    if getattr(tile.TileContext, "_drain_split_patched", False):
        return

    def _drain_and_barrier(self, tick_clock, wait_clock):
        vc = tick_clock.global_clock
        n = len(vc)
        for p in range(n):
            t = vc[p]
            if t <= 0:
                continue
            single = VectorClock([t if i == p else 0 for i in range(n)])
            inst = self.nc.sync.nop(nofuse=True, hint="drain_split_wait")
            wait_clock.add_sem_waits(inst.ins, ScopedClock({None: single}))
        self.nc.sync.drain()
        self.nc.all_engine_barrier()
        assert self.sems is not None
        popped = self.nc._tile_sem_poison_stack.pop()
        assert popped is self._sem_poison
        self.nc.clear_and_free_semaphores(list(self.sems.allocated().values()))
        self.nc.all_engine_barrier()

    tile.TileContext._drain_and_barrier = _drain_and_barrier
    tile.TileContext._drain_split_patched = True


def _patch_to_json_split_waits():
    """This walrus build rejects instructions carrying more than one sem-wait
    ("Too many sync wait commands"). Post-process the BIR JSON: keep one wait
    on each instruction and move extras onto same-engine NoOps inserted just
    before it (identical sync semantics: the engine blocks on the nops first)."""
    if getattr(bass.Bass, "_split_waits_patched", False):
        return
    import json as _json

    orig = bass.Bass.to_json_bytes

    def to_json_bytes(self):
        d = _json.loads(orig(self))
        for fn in d["functions"]:
            for blk in fn["blocks"]:
                out = []
                changed = False
                for ins in blk["instructions"]:
                    si = ins.get("sync_info")
                    waits = (si or {}).get("on_wait") or []
                    if len(waits) > 1:
                        changed = True
                        for k, w in enumerate(waits[:-1]):
                            out.append({
                                "debug": ins.get("debug", 0),
                                "engine": ins["engine"],
                                "ins": [],
                                "name": f"{ins['name']}-w{k}",
                                "opcode": "NoOp",
                                "outs": [],
                                "sync_info": {"on_update": [], "on_wait": [w]},
                                "text_hint": "split_wait",
                            })
                        si["on_wait"] = [waits[-1]]
                    out.append(ins)
                if changed:
                    blk["instructions"] = out
        return _json.dumps(d).encode()

    bass.Bass.to_json_bytes = to_json_bytes
    bass.Bass._split_waits_patched = True


def _bcast_part(ap, n):
    """[1, m] AP -> [n, m] AP broadcasting along partitions (step 0)."""
    apl = ap.ap
    assert apl[0][1] == 1, apl
    return bass.AP(tensor=ap.tensor, offset=ap.offset, ap=[[0, n]] + [list(d) for d in apl[1:]])


def build_nc(reps=1):
    """reps>1 re-emits the whole kernel body back-to-back inside one NEFF —
    used only for timing (amortizes the ~2.5ms axon dispatch cost per
    execution; device time per iteration = slope between two reps values)."""
    nc = bass.Bass()

    x_d = nc.dram_tensor("x", [C, SEQ], F32, kind="ExternalInput")
    xq_d = nc.dram_tensor("xq", [C, NQC], F32, kind="ExternalInput")
    wT_d = nc.dram_tensor("wT", [C, 3 * C], F32, kind="ExternalInput")
    projT_d = nc.dram_tensor("projT", [C, C], F32, kind="ExternalInput")
    pvec_d = nc.dram_tensor("pvec", [128, 8], F32, kind="ExternalInput")
    projbr_d = nc.dram_tensor("projbr", [1, C], F32, kind="ExternalInput")
    G_d = nc.dram_tensor("G", [128, 16], F32, kind="ExternalInput")
    GT_d = nc.dram_tensor("GT", [16, 128], F32, kind="ExternalInput")
    out_d = nc.dram_tensor("out", [C, NQC], F32, kind="ExternalOutput")

    with tile.TileContext(nc) as tc:
        for _rep in range(reps):
            _build_body(nc, tc, x_d, xq_d, wT_d, projT_d, pvec_d, projbr_d,
                        G_d, GT_d, out_d)
    return nc


def _build_body(nc, tc, x_d, xq_d, wT_d, projT_d, pvec_d, projbr_d, G_d, GT_d, out_d):
    if True:
        with (
            tc.tile_pool(name="singles", bufs=1) as singles,
            tc.tile_pool(name="kvq", bufs=1) as kvq,
            tc.tile_pool(name="xs", bufs=2) as xs,
            tc.tile_pool(name="xn", bufs=3) as xnp,
            tc.tile_pool(name="pt", bufs=2) as ptp,
            tc.tile_pool(name="epi", bufs=3) as epi,
            tc.tile_pool(name="outp", bufs=2) as outp,
            tc.tile_pool(name="pg", bufs=4) as pg,
            tc.tile_pool(name="sps", bufs=2, space="PSUM") as sps,
            tc.tile_pool(name="accps", bufs=2, space="PSUM") as accps,
            tc.tile_pool(name="ops", bufs=1, space="PSUM") as ops,
            tc.tile_pool(name="dr", bufs=2, space="DRAM") as drp,
        ):
            # ---- load constants ----
            wT_sb = singles.tile([128, 2, 3 * C], F32, tag="wT")
            nc.sync.dma_start(out=wT_sb[:, 0, :], in_=wT_d[0:128, :])
            nc.sync.dma_start(out=wT_sb[:, 1, :], in_=wT_d[128:256, :])
            projT_sb = singles.tile([128, 2, C], F32, tag="projT")
            nc.sync.dma_start(out=projT_sb[:, 0, :], in_=projT_d[0:128, :])
            nc.sync.dma_start(out=projT_sb[:, 1, :], in_=projT_d[128:256, :])
            pvec_sb = singles.tile([128, 8], F32, tag="pvec")
            nc.sync.dma_start(out=pvec_sb, in_=pvec_d[:, :])
            projbr_sb = singles.tile([1, C], F32, tag="projbr")
            nc.sync.dma_start(out=projbr_sb, in_=projbr_d[:, :])
            G_sb = singles.tile([128, 16], F32, tag="G")
            nc.sync.dma_start(out=G_sb, in_=G_d[:, :])
            GT_sb = singles.tile([16, 128], F32, tag="GT")
            nc.sync.dma_start(out=GT_sb, in_=GT_d[:, :])
            xq_sb = singles.tile([128, 2, NQC], F32, tag="xq")
            nc.sync.dma_start(out=xq_sb[:, 0, :], in_=xq_d[0:128, :])
            nc.sync.dma_start(out=xq_sb[:, 1, :], in_=xq_d[128:256, :])

            # float32r-rounded copies of matmul operands (PE fast mode)
            wT_r = singles.tile([128, 2, 3 * C], F32R, tag="wT_r")
            nc.vector.tensor_copy(out=wT_r[:, 0, :], in_=wT_sb[:, 0, :])
            nc.vector.tensor_copy(out=wT_r[:, 1, :], in_=wT_sb[:, 1, :])
            projT_r = singles.tile([128, 2, C], F32R, tag="projT_r")
            nc.vector.tensor_copy(out=projT_r[:, 0, :], in_=projT_sb[:, 0, :])
            nc.vector.tensor_copy(out=projT_r[:, 1, :], in_=projT_sb[:, 1, :])
            projbr_r = singles.tile([1, C], F32R, tag="projbr_r")
            nc.vector.tensor_copy(out=projbr_r, in_=projbr_sb)
            ones_f = singles.tile([1, 512], F32, tag="ones_f")
            nc.vector.memset(ones_f, 1.0)
            ones_r = singles.tile([1, 512], F32R, tag="ones_r")
            nc.vector.tensor_copy(out=ones_r, in_=ones_f)

            # ---- GroupNorm stats (one pass over x) ----
            stats = singles.tile([128, 2, NST, 6], F32, tag="stats")
            for cc in range(2):
                for bt in range(NST // 2):
                    xt = xs.tile([128, 2 * ST], F32, tag=f"x{cc}")
                    eng = nc.sync if bt % 2 == 0 else nc.gpsimd
                    eng.dma_start(out=xt, in_=x_d[128 * cc:128 * (cc + 1), 2 * ST * bt:2 * ST * (bt + 1)])
                    nc.vector.bn_stats(out=stats[:, cc, 2 * bt, :], in_=xt[:, 0:ST])
                    nc.vector.bn_stats(out=stats[:, cc, 2 * bt + 1, :], in_=xt[:, ST:2 * ST])

            ab_sb = singles.tile([128, 2, 2], F32, tag="ab")  # [:, cc, 0]=a, [:, cc, 1]=b
            for cc in range(2):
                mv = pg.tile([128, 2], F32, tag="mv")
                nc.vector.bn_aggr(out=mv, in_=stats[:, cc, :, :])
                st2 = pg.tile([128, 2], F32, tag="st2")  # (mean, E[x^2])
                nc.vector.tensor_copy(out=st2[:, 0:1], in_=mv[:, 0:1])
                nc.vector.tensor_tensor(out=st2[:, 1:2], in0=mv[:, 0:1], in1=mv[:, 0:1], op=ALU.mult)
                nc.vector.tensor_tensor(out=st2[:, 1:2], in0=st2[:, 1:2], in1=mv[:, 1:2], op=ALU.add)
                gps = accps.tile([128, 512], F32, tag="acc")
                nc.tensor.matmul(gps[0:16, 0:2], lhsT=G_sb, rhs=st2, start=True, stop=True)
                gm = pg.tile([16, 2], F32, tag="gm")  # (mean_g, E2_g)
                nc.vector.tensor_copy(out=gm, in_=gps[0:16, 0:2])
                t16 = pg.tile([16, 1], F32, tag="t16")
                nc.vector.tensor_tensor(out=t16, in0=gm[:, 0:1], in1=gm[:, 0:1], op=ALU.mult)
                nc.vector.tensor_tensor(out=gm[:, 1:2], in0=gm[:, 1:2], in1=t16, op=ALU.subtract)
                # rstd = 1/sqrt(var+eps)
                nc.vector.tensor_scalar_add(out=gm[:, 1:2], in0=gm[:, 1:2], scalar1=EPS)
                nc.scalar.activation(out=gm[:, 1:2], in_=gm[:, 1:2], func=AF.Sqrt)
                nc.vector.reciprocal(out=gm[:, 1:2], in_=gm[:, 1:2])
                mps = accps.tile([128, 512], F32, tag="acc")
                nc.tensor.matmul(mps[0:128, 0:2], lhsT=GT_sb, rhs=gm, start=True, stop=True)
                mr = pg.tile([128, 2], F32, tag="mr")  # (mean_c, rstd_c)
                nc.vector.tensor_copy(out=mr, in_=mps[0:128, 0:2])
                # a = rstd * norm_w ; b = norm_b - mean * a
                nc.vector.tensor_tensor(out=ab_sb[:, cc, 0:1], in0=mr[:, 1:2], in1=pvec_sb[:, 4 + cc:5 + cc], op=ALU.mult)
                t128 = pg.tile([128, 1], F32, tag="t128")
                nc.vector.tensor_tensor(out=t128, in0=mr[:, 0:1], in1=ab_sb[:, cc, 0:1], op=ALU.mult)
                nc.vector.tensor_tensor(out=ab_sb[:, cc, 1:2], in0=pvec_sb[:, 6 + cc:7 + cc], in1=t128, op=ALU.subtract)

            # normalized query tokens (shared by both passes)
            xnq_sb = singles.tile([128, 2, NQC], F32R, tag="xnq")
            for cc in range(2):
                nc.vector.tensor_scalar(
                    out=xnq_sb[:, cc, :], in0=xq_sb[:, cc, :],
                    scalar1=ab_sb[:, cc, 0:1], scalar2=ab_sb[:, cc, 1:2],
                    op0=ALU.mult, op1=ALU.add)

            # persistent per-pass buffers. k/v are split into 4 sub-tiles of 18
            # key-chunks each so pass-1 writes only WAR-wait on the sub-range,
            # letting pass-1 qkv overlap the tail of pass-0 attention.
            KSUB = 18  # key-chunks per sub-tile
            ksubs = [kvq.tile([128, KSUB * 128], F32R, tag=f"k{i}", name=f"k{i}") for i in range(4)]
            vsubs = [kvq.tile([128, KSUB, 130], F32R, tag=f"v{i}", name=f"v{i}") for i in range(4)]
            qsubs = [kvq.tile([128, 512], F32R, tag=f"qw{w}", name=f"qw{w}") for w in range(3)]

            def kslice(r0, kc):
                return ksubs[kc // KSUB][r0:r0 + 64, (kc % KSUB) * 128:(kc % KSUB) * 128 + 128]

            def vslice(kc, hl):
                t = vsubs[kc // KSUB].rearrange("p j (h c) -> p j h c", h=2)
                return t[:, kc % KSUB, hl, :]

            def qslice(r0, q0, qn):
                w = 0 if q0 < 512 else (1 if q0 < 1024 else 2)
                return qsubs[w][r0:r0 + 64, q0 - 512 * w:q0 - 512 * w + qn]

            attn_w = [singles.tile([128, 2, qn], F32, tag=f"attn{w}", name=f"attn{w}")
                      for w, (q0, qn, _, _) in enumerate(WINDOWS)]

            # ones columns of v (col 64 of each head slot); survive both passes
            ones_c = singles.tile([128, 1], F32, tag="ones_c")
            nc.vector.memset(ones_c, 1.0)
            for i in range(4):
                vv = vsubs[i].rearrange("p j (h c) -> p j h c", h=2)
                ones_bc = bass.AP(tensor=ones_c.tensor, offset=ones_c.offset,
                                  ap=[list(ones_c.ap[0]), [0, KSUB], [0, 2], [1, 1]])
                nc.vector.tensor_copy(out=vv[:, :, :, 64:65], in_=ones_bc)

            # ---- software-pipelined emission ----
            # The Tile scheduler prioritizes by program order, so cross-phase
            # overlap requires interleaved *emission*: qkv tiles for pass p+1
            # are emitted between attention S-tile groups of pass p, gated so
            # a k/v sub-tile is only (re)written after every already-emitted
            # reader of it (avoids priority-inverted PSUM slot deadlocks).
            xbigs = {}

            def emit_qkv_tile(p, st):
                s0 = ST * st
                xn_t = xnp.tile([128, 2, ST], F32R, tag="xn", name="xn_t")
                for cc in range(2):
                    if st % 2 == 0:
                        xbig = xs.tile([128, 2 * ST], F32, tag=f"x{cc}", name=f"xbig{cc}")
                        xbigs[cc] = xbig
                        eng = nc.sync if (st // 2) % 2 == 0 else nc.gpsimd
                        eng.dma_start(out=xbig, in_=x_d[128 * cc:128 * (cc + 1), s0:s0 + 2 * ST])
                    xt = xbigs[cc][:, (st % 2) * ST:(st % 2) * ST + ST]
                    nc.vector.tensor_scalar(
                        out=xn_t[:, cc, :], in0=xt,
                        scalar1=ab_sb[:, cc, 0:1], scalar2=ab_sb[:, cc, 1:2],
                        op0=ALU.mult, op1=ALU.add)
                # kT rows 256+128p ..  (2 heads x 64)
                kps = accps.tile([128, 512], F32, tag="acc", name="kps")
                for cc in range(2):
                    nc.tensor.matmul(
                        kps, lhsT=wT_r[:, cc, C + 128 * p:C + 128 * p + 128],
                        rhs=xn_t[:, cc, :], start=(cc == 0), stop=(cc == 1))
                c0 = 4 * st
                bnds = (((c0, c0 + 4),) if (c0 // KSUB == (c0 + 3) // KSUB)
                        else ((c0, (c0 // KSUB + 1) * KSUB), ((c0 // KSUB + 1) * KSUB, c0 + 4)))
                for (clo, chi) in bnds:
                    sub = ksubs[clo // KSUB]
                    nc.vector.tensor_scalar_add(
                        out=sub[:, (clo % KSUB) * 128:(clo % KSUB) * 128 + (chi - clo) * 128],
                        in0=kps[:, (clo - c0) * 128:(chi - c0) * 128],
                        scalar1=pvec_sb[:, 2 + p:3 + p])
                # v cols 512+128p ..; 4 token sub-chunks
                vps = accps.tile([128, 512], F32, tag="acc", name="vps")
                for mc in range(4):
                    for cc in range(2):
                        nc.tensor.matmul(
                            vps[:, 128 * mc:128 * (mc + 1)],
                            lhsT=xn_t[:, cc, 128 * mc:128 * (mc + 1)],
                            rhs=wT_r[:, cc, 2 * C + 128 * p:2 * C + 128 * p + 128],
                            start=(cc == 0), stop=(cc == 1))
                vpsv = vps.rearrange("p (j h c) -> p j h c", j=4, h=2)
                for (clo, chi) in bnds:
                    vv = vsubs[clo // KSUB].rearrange("p j (h c) -> p j h c", h=2)
                    nc.vector.tensor_copy(
                        out=vv[:, clo % KSUB:clo % KSUB + (chi - clo), :, 0:64],
                        in_=vpsv[:, clo - c0:chi - c0, :, :])

            def emit_q(p, w):
                qt0, qtn = ((0, 512), (512, 512), (1024, 128))[w]
                qps = accps.tile([128, 512], F32, tag="acc", name="qps")
                for cc in range(2):
                    nc.tensor.matmul(
                        qps[:, 0:qtn], lhsT=wT_r[:, cc, 128 * p:128 * p + 128],
                        rhs=xnq_sb[:, cc, qt0:qt0 + qtn], start=(cc == 0), stop=(cc == 1))
                nc.vector.tensor_scalar_add(out=qsubs[w][:, 0:qtn],
                                            in0=qps[:, 0:qtn], scalar1=pvec_sb[:, p:p + 1])

            def gen_attention(p, w):
                """Generator: one yield per S-tile group; epilogues at the end.
                Both heads' key-chunks are interleaved in one stream so adjacent
                K=64 QK matmuls hit disjoint PE row groups (run concurrently).
                QK is emitted one group AHEAD of exp/PV: PE executes in order,
                so group g's PV (gated on exp g) must come after group g+1's
                QK (ready immediately) or PE idles every group waiting on the
                Activation engine — which also HAM-downclocks the PE."""
                q0, qn, kc0, nch = WINDOWS[w]
                o_t = {hl: ops.tile([128, 512], F32, tag=f"o{hl}", name=f"o{hl}") for hl in range(2)}
                stream = [(hl, kc0 + c) for c in range(nch) for hl in range(2)]
                groups = [stream[i:i + SCHUNK] for i in range(0, len(stream), SCHUNK)]
                s_tiles = {}

                def emit_qk(g):
                    # each QK matmul output must start on a PSUM bank boundary
                    s_ps = sps.tile([128, 2, 512], F32, tag="s", name="s_ps")
                    for j, (hl, kc) in enumerate(groups[g]):
                        nc.tensor.matmul(
                            s_ps[:, j, 0:qn],
                            lhsT=kslice(64 * hl, kc),
                            rhs=qslice(64 * hl, q0, qn),
                            start=True, stop=True)
                    s_tiles[g] = s_ps

                emit_qk(0)
                for g in range(len(groups)):
                    if g + 1 < len(groups):
                        emit_qk(g + 1)
                    items = groups[g]
                    m = len(items)
                    s_ps = s_tiles.pop(g)
                    pt = ptp.tile([128, 2 * 512], F32R, tag="p", name="pt")
                    ptv = pt[:, 0:m * qn].rearrange("p (j c) -> p j c", j=m)
                    nc.scalar.activation(out=ptv, in_=s_ps[:, 0:m, 0:qn], func=AF.Exp, scale=SCALE)
                    for j, (hl, kc) in enumerate(items):
                        nc.tensor.matmul(
                            o_t[hl][0:65, 0:qn],
                            lhsT=vslice(kc, hl),
                            rhs=pt[:, qn * j:qn * (j + 1)],
                            start=(kc == kc0), stop=(kc == kc0 + nch - 1))
                    yield
                for hl in range(2):
                    # epilogue: copy O out of PSUM (frees the bank fast),
                    # normalize by the broadcast reciprocal of the sums row
                    osb = epi.tile([65, 512], F32, tag="osb", name="osb")
                    nc.vector.tensor_copy(out=osb[:, 0:qn], in_=o_t[hl][0:65, 0:qn])
                    rec = epi.tile([1, 512], F32, tag="rec", name="rec")
                    nc.vector.reciprocal(out=rec[0:1, 0:qn], in_=osb[64:65, 0:qn])
                    rd = drp.tile([1, 512], F32, tag="rd", name="rd")
                    nc.gpsimd.dma_start(out=rd[0:1, 0:qn], in_=rec[0:1, 0:qn])
                    recb = epi.tile([64, 512], F32, tag="recb", name="recb")
                    nc.gpsimd.dma_start(out=recb[0:64, 0:qn], in_=_bcast_part(rd[0:1, 0:qn], 64))
                    if hl == 0:
                        nc.vector.tensor_tensor(
                            out=attn_w[w][0:64, p, 0:qn],
                            in0=osb[0:64, 0:qn], in1=recb[0:64, 0:qn], op=ALU.mult)
                    else:
                        at = epi.tile([64, 512], F32, tag="at", name="at")
                        nc.vector.tensor_tensor(
                            out=at[:, 0:qn], in0=osb[0:64, 0:qn], in1=recb[0:64, 0:qn], op=ALU.mult)
                        nc.sync.dma_start(out=attn_w[w][64:128, p, 0:qn], in_=at[:, 0:qn])

            def drain(g):
                for _ in g:
                    pass

            # chunk c's last pass-p attention reader: (window, group index).
            # chunk c appears in window w's hl-interleaved stream at item
            # positions 2*(c-kc0) and 2*(c-kc0)+1; groups are SCHUNK items.
            def last_reader(c):
                if c < 16:
                    return (0, (2 * c + 1) // SCHUNK)
                if c < 48:
                    return (1, (2 * (c - 16) + 1) // SCHUNK)
                return (2, (2 * (c - 48) + 1) // SCHUNK)

            def sts_after(w, g):
                """qkv seq-tiles whose chunks' last readers are exactly (w, g)."""
                out = []
                for st in range(NST):
                    lrs = [last_reader(c) for c in range(4 * st, 4 * st + 4)]
                    if max(lrs) == (w, g):
                        out.append(st)
                return out

            # ---- pass 0: qkv interleaved with W0 attention ----
            for w in range(3):
                emit_q(0, w)
            g0 = gen_attention(0, 0)
            n_emitted = 0
            n_groups_w0 = (2 * WINDOWS[0][3] + SCHUNK - 1) // SCHUNK
            for st in range(NST):
                emit_qkv_tile(0, st)
                # iteration g of the generator emits QK(g+1) (one group ahead),
                # which reads chunks up to (SCHUNK*(g+2)-1)//2; advance while
                # that is <= the last key-chunk written so far (4*st+3)
                while n_emitted < n_groups_w0 and (SCHUNK * (n_emitted + 2) - 1) // 2 <= 4 * st + 3:
                    next(g0)
                    n_emitted += 1
            drain(g0)

            # ---- pass 0 W1/W2 attention interleaved with pass-1 qkv ----
            for st in range(4):  # chunks 0..15: last pass-0 reader was W0 (done)
                emit_qkv_tile(1, st)
            emit_q(1, 0)  # qw0: WAR on W0-pass0 reads (all emitted)
            g1 = gen_attention(0, 1)
            g = 0
            while True:
                try:
                    next(g1)
                except StopIteration:
                    break
                for st in sts_after(1, g):
                    emit_qkv_tile(1, st)
                g += 1
            emit_q(1, 1)
            g2 = gen_attention(0, 2)
            g = 0
            while True:
                try:
                    next(g2)
                except StopIteration:
                    break
                for st in sts_after(2, g):
                    emit_qkv_tile(1, st)
                g += 1
            emit_q(1, 2)

            # ---- pass 1 attention, each window's projection emitted right
            # after it so proj overlaps the next window's attention ----
            for w in range(3):
                drain(gen_attention(1, w))
                qt0, qtn, _, _ = WINDOWS[w]
                attn_rw = singles.tile([128, 2, qtn], F32R, tag=f"attn_r{w}", name=f"attn_r{w}")
                nc.vector.tensor_copy(out=attn_rw[:, 0, :], in_=attn_w[w][:, 0, :])
                nc.vector.tensor_copy(out=attn_rw[:, 1, :], in_=attn_w[w][:, 1, :])
                for mc in range(2):
                    pp = accps.tile([128, 512], F32, tag="acc")
                    nc.tensor.matmul(pp[:, 0:qtn], lhsT=projbr_r[0:1, 128 * mc:128 * (mc + 1)],
                                     rhs=ones_r[0:1, 0:qtn], start=True, stop=False)
                    for cc in range(2):
                        nc.tensor.matmul(pp[:, 0:qtn], lhsT=projT_r[:, cc, 128 * mc:128 * (mc + 1)],
                                         rhs=attn_rw[:, cc, 0:qtn],
                                         start=False, stop=(cc == 1))
                    ot = outp.tile([128, 512], F32, tag="ot")
                    nc.vector.tensor_tensor(out=ot[:, 0:qtn], in0=pp[:, 0:qtn],
                                            in1=xq_sb[:, mc, qt0:qt0 + qtn], op=ALU.add)
                    nc.sync.dma_start(out=out_d[128 * mc:128 * (mc + 1), qt0:qt0 + qtn], in_=ot[:, 0:qtn])


def make_inputs(x, norm_w, norm_b, qkv_w, qkv_b, proj_w, proj_b):
    """Host-side prep: full-input numpy -> per-core in_maps."""
    x2 = np.ascontiguousarray(np.asarray(x, np.float32).reshape(C, SEQ))
    qkv_w = np.asarray(qkv_w, np.float32)
    qkv_b = np.asarray(qkv_b, np.float32)
    proj_w = np.asarray(proj_w, np.float32)
    proj_b = np.asarray(proj_b, np.float32)
    norm_w = np.asarray(norm_w, np.float32)
    norm_b = np.asarray(norm_b, np.float32)

    wT = np.ascontiguousarray(qkv_w.T)
    projT = np.ascontiguousarray(proj_w.T)
    # v-bias folds into the projection bias: proj(attn + bv) = proj(attn) + proj_w @ bv
    projbr = (proj_b + proj_w @ qkv_b[2 * C:3 * C]).reshape(1, C).astype(np.float32)
    pvec = np.stack([
        qkv_b[0:128], qkv_b[128:256],          # q bias pass0/1
        qkv_b[C:C + 128], qkv_b[C + 128:2 * C],  # k bias pass0/1
        norm_w[0:128], norm_w[128:256],
        norm_b[0:128], norm_b[128:256],
    ], axis=1).astype(np.float32)
    cidx = np.arange(128)
    gidx = np.arange(16)
    G = ((cidx[:, None] // 8) == gidx[None, :]).astype(np.float32) / 8.0
    GT = np.ascontiguousarray(G.T * 8.0)

    common = dict(x=x2, wT=wT, projT=projT, pvec=pvec, projbr=projbr, G=G, GT=GT)
    in_maps = []
    cols = []
    for i in range(NCORES):
        ci = np.concatenate([
            np.arange(512 * i, 512 * (i + 1)),
            np.arange(4096 + 512 * i, 4096 + 512 * (i + 1)),
            np.arange(8192 + 128 * i, 8192 + 128 * (i + 1)),
        ])
        cols.append(ci)
        m = dict(common)
        m["xq"] = np.ascontiguousarray(x2[:, ci])
        in_maps.append(m)
    return in_maps, cols


_NC_CACHE = {}


def kernel(x, norm_w, norm_b, qkv_w, qkv_b, proj_w, proj_b):
    from concourse.bass_utils import run_bass_kernel_spmd

    _patch_tile_drain()
    _patch_to_json_split_waits()
    in_maps, cols = make_inputs(x, norm_w, norm_b, qkv_w, qkv_b, proj_w, proj_b)
    if "nc" not in _NC_CACHE:
        _NC_CACHE["nc"] = build_nc()
    nc = _NC_CACHE["nc"]
    res = run_bass_kernel_spmd(nc, in_maps, core_ids=list(range(NCORES)))
    out = np.zeros((C, SEQ), np.float32)
    for i in range(NCORES):
        out[:, cols[i]] = res.results[i]["out"]
    return out.reshape(1, C, 96, 96)



# revision 9
# speedup vs baseline: 1.3460x; 1.3460x over previous
"""Trainium2 Bass kernel for nn_AttentionBlock (GroupNorm + windowed MHA + proj + residual).

Contract: kernel(**inputs) takes FULL unsharded inputs (as from reference.setup_inputs())
and returns the FULL output [1, 256, 96, 96] float32.

Sharding: sequence-parallel over query positions across 8 cores. Each core gets a
uniform slice of each of the 3 reference attention windows:
  W0: q[512i   : 512(i+1)]    attends kv[0    : 6144]
  W1: q[4096+512i : ...]      attends kv[2048 : 9216]
  W2: q[8192+128i : ...]      attends kv[6144 : 9216]
All 4 heads for those queries are computed locally, so the output projection and
residual are local too. Every core redundantly computes GroupNorm stats and the
full-sequence K/V (its key windows span the whole sequence).

Phase structure (single core):
  1. stats: x [256,9216] f32 streamed in on 4 DMA queues; bn_stats on DVE;
     ScalarE copies each tile to a resident fp16 buffer (xn).
  2. GroupNorm reduce -> per-channel a,b (tiny PE matmuls against group maps).
  3. xn normalized IN PLACE on DVE (fp16, 2x/4x DVE modes); xq -> xnq fp16.
  4. qkv for ALL 4 heads once: K^T [2 head-pairs x 128, seq] fp16, V [token-chunk,
     head, 64|1] fp16 (65th col ones -> softmax sums fall out of the PV matmul),
     q per window fp16. All matmul operands fp16 (PE streams 2-byte lhsT faster;
     fp32 PSUM accumulation keeps precision; rel err ~1e-3 << 2e-2 gate).
  5. Attention sweep per head-pair hp: for each window, stream key-chunks in
     SCHUNK=2 groups: QK pair (2 heads interleaved on PE row-tiles 0/64 -> they
     run concurrently), exp on ScalarE (PSUM->SBUF fp16), PV pair accumulating
     O^T[65, qn] per head in PSUM. QK is emitted ONE GROUP AHEAD of exp/PV:
     the PE executes in order, so group g's PV (gated on exp g) must sit after
     group g+1's QK or the PE idles every group and HAM-downclocks.
     Epilogue per window: sums row -> PE ones-broadcast -> DVE reciprocal+mult
     -> attn [64, head, q] fp16 (everything stays on partitions 0:64 so no
     partition-moving DMAs). Epilogues/projections are injected into the next
     window's group loop so they overlap attention instead of serializing.
  6. Projection per window (after sweep 1): K=64 matmuls per head against a
     head-major projTh layout + rank-1 bias matmul + residual, DMA out.
"""

import numpy as np

import concourse.bass as bass
import concourse.tile as tile
from concourse import mybir
from concourse.vector_clock import ScopedClock, VectorClock

F32 = mybir.dt.float32
F32R = mybir.dt.float32r
F16 = mybir.dt.float16
AF = mybir.ActivationFunctionType
ALU = mybir.AluOpType

C = 256
SEQ = 9216
NCORES = 8
HEADS = 4
D = 64
EPS = 1e-5
SCALE = 0.125  # 1/sqrt(64)
NQC = 1152  # queries per core
ST = 512  # seq tile for qkv
NST = SEQ // ST  # 18
NCH = SEQ // 128  # 72 key chunks
# windows: (q_off, q_len, key_chunk0, n_key_chunks)
WINDOWS = [(0, 512, 0, 48), (512, 512, 16, 56), (1024, 128, 48, 24)]
SCHUNK = 2  # key-chunk items batched per exp ACTIVATE (2 PSUM banks)


def _patch_tile_drain():
    """This container's walrus rejects >1 sem wait on one sync CTRL instruction
    ("Too many sync wait commands"). Split the TileContext-exit drain's waits
    into one-wait-per-nop instructions."""
    if getattr(tile.TileContext, "_drain_split_patched", False):
        return

    def _drain_and_barrier(self, tick_clock, wait_clock):
        vc = tick_clock.global_clock
        n = len(vc)
        for p in range(n):
            t = vc[p]
            if t <= 0:
                continue
            single = VectorClock([t if i == p else 0 for i in range(n)])
            inst = self.nc.sync.nop(nofuse=True, hint="drain_split_wait")
            wait_clock.add_sem_waits(inst.ins, ScopedClock({None: single}))
        self.nc.sync.drain()
        self.nc.all_engine_barrier()
        assert self.sems is not None
        popped = self.nc._tile_sem_poison_stack.pop()
        assert popped is self._sem_poison
        self.nc.clear_and_free_semaphores(list(self.sems.allocated().values()))
        self.nc.all_engine_barrier()

    tile.TileContext._drain_and_barrier = _drain_and_barrier
    tile.TileContext._drain_split_patched = True


def _patch_to_json_split_waits():
    """This walrus build rejects instructions carrying more than one sem-wait
    ("Too many sync wait commands"). Post-process the BIR JSON: keep one wait
    on each instruction and move extras onto same-engine NoOps inserted just
    before it (identical sync semantics: the engine blocks on the nops first)."""
    if getattr(bass.Bass, "_split_waits_patched", False):
        return
    import json as _json

    orig = bass.Bass.to_json_bytes

    def to_json_bytes(self):
        d = _json.loads(orig(self))
        for fn in d["functions"]:
            for blk in fn["blocks"]:
                out = []
                changed = False
                for ins in blk["instructions"]:
                    si = ins.get("sync_info")
                    waits = (si or {}).get("on_wait") or []
                    if len(waits) > 1:
                        changed = True
                        for k, w in enumerate(waits[:-1]):
                            out.append({
                                "debug": ins.get("debug", 0),
                                "engine": ins["engine"],
                                "ins": [],
                                "name": f"{ins['name']}-w{k}",
                                "opcode": "NoOp",
                                "outs": [],
                                "sync_info": {"on_update": [], "on_wait": [w]},
                                "text_hint": "split_wait",
                            })
                        si["on_wait"] = [waits[-1]]
                    out.append(ins)
                if changed:
                    blk["instructions"] = out
        return _json.dumps(d).encode()

    bass.Bass.to_json_bytes = to_json_bytes
    bass.Bass._split_waits_patched = True


def declare_drams(nc):
    d = {}
    d["x"] = nc.dram_tensor("x", [C, SEQ], F32, kind="ExternalInput")
    d["xq"] = nc.dram_tensor("xq", [C, NQC], F32, kind="ExternalInput")
    d["wT"] = nc.dram_tensor("wT", [C, 3 * C], F32, kind="ExternalInput")
    d["projTh"] = nc.dram_tensor("projTh", [64, 4 * C], F32, kind="ExternalInput")
    d["pvec"] = nc.dram_tensor("pvec", [128, 8], F32, kind="ExternalInput")
    d["projbr"] = nc.dram_tensor("projbr", [1, C], F32, kind="ExternalInput")
    d["G"] = nc.dram_tensor("G", [128, 16], F32, kind="ExternalInput")
    d["GT"] = nc.dram_tensor("GT", [16, 128], F32, kind="ExternalInput")
    d["out"] = nc.dram_tensor("out", [C, NQC], F32, kind="ExternalOutput")
    return d


def build_nc(reps=1):
    """reps>1 re-emits the whole kernel body back-to-back inside one NEFF —
    used only for timing (amortizes the ~2.5ms axon dispatch cost per
    execution; device time per iteration = slope between two reps values)."""
    nc = bass.Bass()
    d = declare_drams(nc)
    with tile.TileContext(nc) as tc:
        for _rep in range(reps):
            _build_body(nc, tc, d)
    return nc


def _build_body(nc, tc, d):
    x_d, xq_d, wT_d, projTh_d = d["x"], d["xq"], d["wT"], d["projTh"]
    pvec_d, projbr_d, G_d, GT_d, out_d = d["pvec"], d["projbr"], d["G"], d["GT"], d["out"]

    with (
        tc.tile_pool(name="singles", bufs=1) as singles,
        tc.tile_pool(name="xs", bufs=4) as xs,
        tc.tile_pool(name="pt", bufs=2) as ptp,
        tc.tile_pool(name="epi", bufs=2) as epi,
        tc.tile_pool(name="outp", bufs=2) as outp,
        tc.tile_pool(name="pg", bufs=4) as pg,
        tc.tile_pool(name="sps", bufs=2, space="PSUM") as sps,
        tc.tile_pool(name="accps", bufs=2, space="PSUM") as accps,
        tc.tile_pool(name="ops", bufs=1, space="PSUM") as ops,
    ):
        # ---- constants: DMA + fp16 conversion (staging pool freed after) ----
        pvec_sb = singles.tile([128, 8], F32, tag="pvec")
        nc.sync.dma_start(out=pvec_sb, in_=pvec_d[:, :])
        G_sb = singles.tile([128, 16], F32, tag="G")
        nc.sync.dma_start(out=G_sb, in_=G_d[:, :])
        GT_sb = singles.tile([16, 128], F32, tag="GT")
        nc.sync.dma_start(out=GT_sb, in_=GT_d[:, :])
        xq_sb = singles.tile([128, 2, NQC], F32, tag="xq")
        nc.sync.dma_start(out=xq_sb[:, 0, :], in_=xq_d[0:128, :])
        nc.sync.dma_start(out=xq_sb[:, 1, :], in_=xq_d[128:256, :])

        wT_h = singles.tile([128, 2, 3 * C], F16, tag="wT_h")
        projTh_h = singles.tile([64, 4, C], F16, tag="projTh_h")
        projbr_h = singles.tile([1, C], F16, tag="projbr_h")
        with tc.tile_pool(name="wstage", bufs=1) as wstage:
            ws = wstage.tile([128, 2, 3 * C], F32, tag="ws")
            nc.sync.dma_start(out=ws[:, 0, :], in_=wT_d[0:128, :])
            nc.sync.dma_start(out=ws[:, 1, :], in_=wT_d[128:256, :])
            nc.vector.tensor_copy(out=wT_h[:, 0, :], in_=ws[:, 0, :])
            nc.vector.tensor_copy(out=wT_h[:, 1, :], in_=ws[:, 1, :])
            ps = wstage.tile([64, 4 * C], F32, tag="ps")
            nc.sync.dma_start(out=ps, in_=projTh_d[:, :])
            nc.vector.tensor_copy(out=projTh_h.rearrange("p h c -> p (h c)"), in_=ps)
            pbs = wstage.tile([1, C], F32, tag="pbs")
            nc.sync.dma_start(out=pbs, in_=projbr_d[:, :])
            nc.vector.tensor_copy(out=projbr_h, in_=pbs)

        ones_h = singles.tile([1, 512], F16, tag="ones_h")
        nc.vector.memset(ones_h, 1.0)
        ones_f32 = singles.tile([128, 64], F32, tag="ones_f32")
        nc.vector.memset(ones_f32, 1.0)
        ones_c = singles.tile([128, 1], F16, tag="ones_c")
        nc.vector.memset(ones_c, 1.0)

        # ---- persistent fp16 state ----
        xn = singles.tile([128, 2, SEQ], F16, tag="xn")          # raw x, then normalized
        k_res = singles.tile([128, 2, SEQ], F16, tag="k_res")    # [2-head rows, hp, token]
        v_res = singles.tile([128, NCH, 2, 130], F16, tag="v_res")  # [tok%128, chunk, hp, 2x(64|1)]
        q_res = singles.tile([128, 2, 3, 512], F16, tag="q_res")  # [2-head rows, hp, window, q]
        attn_sb = singles.tile([64, HEADS, NQC], F16, tag="attn")  # [dim, head, q]
        xnq = singles.tile([128, 2, NQC], F16, tag="xnq")

        # ones columns of v (col 64 of each head slot) — written once
        v5 = v_res.rearrange("p ch hp (hl c) -> p ch hp hl c", hl=2)
        ones_bc = bass.AP(tensor=ones_c.tensor, offset=ones_c.offset,
                          ap=[list(ones_c.ap[0]), [0, NCH], [0, 2], [0, 2], [1, 1]])
        nc.vector.tensor_copy(out=v5[:, :, :, :, 64:65], in_=ones_bc)

        # ---- phase 1: x load (4 DMA queues) + bn_stats + fp16 copy ----
        stats = singles.tile([128, 2, NST, 6], F32, tag="stats")
        qeng = [nc.gpsimd, nc.scalar, nc.sync]
        for bt in range(NST // 2):  # 9 big tiles of [128, 1024] per cc
            for cc in range(2):
                xt = xs.tile([128, 2 * ST], F32, tag="xbig", name="xbig")
                qeng[(2 * bt + cc) % 3].dma_start(
                    out=xt, in_=x_d[128 * cc:128 * (cc + 1), 2 * ST * bt:2 * ST * (bt + 1)])
                nc.vector.bn_stats(out=stats[:, cc, 2 * bt, :], in_=xt[:, 0:ST])
                nc.vector.bn_stats(out=stats[:, cc, 2 * bt + 1, :], in_=xt[:, ST:2 * ST])
                nc.scalar.activation(out=xn[:, cc, 2 * ST * bt:2 * ST * (bt + 1)],
                                     in_=xt, func=AF.Copy)

        # ---- phase 2: GroupNorm stats -> per-channel a, b ----
        ab_sb = singles.tile([128, 2, 2], F32, tag="ab")  # [:, cc, 0]=a, [:, cc, 1]=b
        for cc in range(2):
            mv = pg.tile([128, 2], F32, tag="mv")
            nc.vector.bn_aggr(out=mv, in_=stats[:, cc, :, :])
            st2 = pg.tile([128, 2], F32, tag="st2")  # (mean, E[x^2])
            nc.vector.tensor_copy(out=st2[:, 0:1], in_=mv[:, 0:1])
            nc.vector.tensor_tensor(out=st2[:, 1:2], in0=mv[:, 0:1], in1=mv[:, 0:1], op=ALU.mult)
            nc.vector.tensor_tensor(out=st2[:, 1:2], in0=st2[:, 1:2], in1=mv[:, 1:2], op=ALU.add)
            gps = accps.tile([128, 512], F32, tag="acc")
            nc.tensor.matmul(gps[0:16, 0:2], lhsT=G_sb, rhs=st2, start=True, stop=True)
            gm = pg.tile([16, 2], F32, tag="gm")  # (mean_g, E2_g)
            nc.vector.tensor_copy(out=gm, in_=gps[0:16, 0:2])
            t16 = pg.tile([16, 1], F32, tag="t16")
            nc.vector.tensor_tensor(out=t16, in0=gm[:, 0:1], in1=gm[:, 0:1], op=ALU.mult)
            nc.vector.tensor_tensor(out=gm[:, 1:2], in0=gm[:, 1:2], in1=t16, op=ALU.subtract)
            # rstd = 1/sqrt(var+eps)
            nc.vector.tensor_scalar_add(out=gm[:, 1:2], in0=gm[:, 1:2], scalar1=EPS)
            nc.scalar.activation(out=gm[:, 1:2], in_=gm[:, 1:2], func=AF.Sqrt)
            nc.vector.reciprocal(out=gm[:, 1:2], in_=gm[:, 1:2])
            mps = accps.tile([128, 512], F32, tag="acc")
            nc.tensor.matmul(mps[0:128, 0:2], lhsT=GT_sb, rhs=gm, start=True, stop=True)
            mr = pg.tile([128, 2], F32, tag="mr")  # (mean_c, rstd_c)
            nc.vector.tensor_copy(out=mr, in_=mps[0:128, 0:2])
            # a = rstd * norm_w ; b = norm_b - mean * a
            nc.vector.tensor_tensor(out=ab_sb[:, cc, 0:1], in0=mr[:, 1:2], in1=pvec_sb[:, 4 + cc:5 + cc], op=ALU.mult)
            t128 = pg.tile([128, 1], F32, tag="t128")
            nc.vector.tensor_tensor(out=t128, in0=mr[:, 0:1], in1=ab_sb[:, cc, 0:1], op=ALU.mult)
            nc.vector.tensor_tensor(out=ab_sb[:, cc, 1:2], in0=pvec_sb[:, 6 + cc:7 + cc], in1=t128, op=ALU.subtract)

        # ---- phase 3: normalize xn in place; xq -> xnq ----
        for cc in range(2):
            half = SEQ // 2
            for hh in range(2):  # split so DVE chunks interleave with other startup work
                nc.vector.tensor_scalar(
                    out=xn[:, cc, hh * half:(hh + 1) * half],
                    in0=xn[:, cc, hh * half:(hh + 1) * half],
                    scalar1=ab_sb[:, cc, 0:1], scalar2=ab_sb[:, cc, 1:2],
                    op0=ALU.mult, op1=ALU.add)
            nc.vector.tensor_scalar(
                out=xnq[:, cc, :], in0=xq_sb[:, cc, :],
                scalar1=ab_sb[:, cc, 0:1], scalar2=ab_sb[:, cc, 1:2],
                op0=ALU.mult, op1=ALU.add)

        # ---- phase 4: qkv for all 4 heads ----
        def emit_qkv_tile(st):
            s0 = ST * st
            for kb in range(2):  # k rows [128kb:128kb+128] = head-pair kb
                kps = accps.tile([128, 512], F32, tag="acc", name="kps")
                for cc in range(2):
                    nc.tensor.matmul(
                        kps, lhsT=wT_h[:, cc, C + 128 * kb:C + 128 * kb + 128],
                        rhs=xn[:, cc, s0:s0 + ST], start=(cc == 0), stop=(cc == 1))
                nc.vector.tensor_scalar_add(
                    out=k_res[:, kb, s0:s0 + ST], in0=kps,
                    scalar1=pvec_sb[:, 2 + kb:3 + kb])
            for mc in range(4):  # token sub-chunks of 128
                vps = accps.tile([128, 512], F32, tag="acc", name="vps")
                for cc in range(2):
                    nc.tensor.matmul(
                        vps[:, 0:256],
                        lhsT=xn[:, cc, s0 + 128 * mc:s0 + 128 * (mc + 1)],
                        rhs=wT_h[:, cc, 2 * C:3 * C],
                        start=(cc == 0), stop=(cc == 1))
                ch = 4 * st + mc
                vpsv = vps[:, 0:256].rearrange("p (hp hl c) -> p hp hl c", hp=2, hl=2)
                nc.vector.tensor_copy(out=v5[:, ch, :, :, 0:64], in_=vpsv)

        def emit_q():
            for w in range(3):
                qn = WINDOWS[w][1]
                for kb in range(2):
                    qps = accps.tile([128, 512], F32, tag="acc", name="qps")
                    for cc in range(2):
                        nc.tensor.matmul(
                            qps[:, 0:qn], lhsT=wT_h[:, cc, 128 * kb:128 * kb + 128],
                            rhs=xnq[:, cc, 512 * w:512 * w + qn], start=(cc == 0), stop=(cc == 1))
                    nc.vector.tensor_scalar_add(out=q_res[:, kb, w, 0:qn],
                                                in0=qps[:, 0:qn], scalar1=pvec_sb[:, kb:kb + 1])

        emit_q()
        for st in range(NST):
            emit_qkv_tile(st)

        # ---- attention ----
        def kslice(r0, hp, kc):
            return k_res[r0:r0 + 64, hp, 128 * kc:128 * kc + 128]

        def vslice(kc, hp, hl):
            return v5[:, kc, hp, hl, :]

        def run_attention(hp, w, inject=None):
            """Stream window w's key-chunks for head-pair hp. inject maps
            group index -> [fn] emitted right after that group (epilogues /
            projections of earlier windows, so they overlap this window)."""
            inject = inject or {}
            q0, qn, kc0, nch = WINDOWS[w]
            o_t = {hl: ops.tile([128, 512], F32, tag=f"o{hl}", name=f"o{hl}") for hl in range(2)}
            stream = [(hl, kc0 + c) for c in range(nch) for hl in range(2)]
            groups = [stream[i:i + SCHUNK] for i in range(0, len(stream), SCHUNK)]
            s_tiles = {}

            def emit_qk(g):
                # each QK matmul output must start on a PSUM bank boundary
                s_ps = sps.tile([128, 2, 512], F32, tag="s", name="s_ps")
                for j, (hl, kc) in enumerate(groups[g]):
                    nc.tensor.matmul(
                        s_ps[:, j, 0:qn],
                        lhsT=kslice(64 * hl, hp, kc),
                        rhs=q_res[64 * hl:64 * hl + 64, hp, w, 0:qn],
                        start=True, stop=True)
                s_tiles[g] = s_ps

            for fn in inject.pop(-1, []):
                fn()
            emit_qk(0)
            for g in range(len(groups)):
                if g + 1 < len(groups):
                    emit_qk(g + 1)
                items = groups[g]
                m = len(items)
                s_ps = s_tiles.pop(g)
                pt = ptp.tile([128, 2 * 512], F16, tag="p", name="pt")
                ptv = pt[:, 0:m * qn].rearrange("p (j c) -> p j c", j=m)
                nc.scalar.activation(out=ptv, in_=s_ps[:, 0:m, 0:qn], func=AF.Exp, scale=SCALE)
                for j, (hl, kc) in enumerate(items):
                    nc.tensor.matmul(
                        o_t[hl][0:65, 0:qn],
                        lhsT=vslice(kc, hp, hl),
                        rhs=pt[:, qn * j:qn * (j + 1)],
                        start=(kc == kc0), stop=(kc == kc0 + nch - 1))
                for fn in inject.pop(g, []):
                    fn()
            return o_t

        def make_epilogue(hp, w, o_t):
            """O^T[0:64] / O^T[64] -> attn[0:64, head, q]  (all on partitions 0:64)."""
            q0, qn, _, _ = WINDOWS[w]

            def fn():
                for hl in range(2):
                    osb = epi.tile([65, 512], F32, tag="osb", name="osb")
                    nc.vector.tensor_copy(out=osb[:, 0:qn], in_=o_t[hl][0:65, 0:qn])
                    # sums row -> broadcast [64, qn] on PE (fp32 rank-1), then
                    # reciprocal+mult on DVE across all 64 lanes at once
                    bps = accps.tile([128, 512], F32, tag="acc", name="bps")
                    nc.tensor.matmul(bps[0:64, 0:qn], lhsT=ones_f32[64:65, :],
                                     rhs=osb[64:65, 0:qn], start=True, stop=True)
                    recb = epi.tile([64, 512], F32, tag="recb", name="recb")
                    nc.vector.reciprocal(out=recb[:, 0:qn], in_=bps[0:64, 0:qn])
                    nc.vector.tensor_tensor(
                        out=attn_sb[:, 2 * hp + hl, q0:q0 + qn],
                        in0=osb[0:64, 0:qn], in1=recb[:, 0:qn], op=ALU.mult)
            return fn

        def make_proj(w):
            q0, qn, _, _ = WINDOWS[w]

            def fn():
                for mc in range(2):
                    pp = accps.tile([128, 512], F32, tag="acc", name="pp")
                    nc.tensor.matmul(pp[:, 0:qn], lhsT=projbr_h[0:1, 128 * mc:128 * (mc + 1)],
                                     rhs=ones_h[0:1, 0:qn], start=True, stop=False)
                    for h in range(HEADS):
                        nc.tensor.matmul(pp[:, 0:qn], lhsT=projTh_h[:, h, 128 * mc:128 * (mc + 1)],
                                         rhs=attn_sb[:, h, q0:q0 + qn],
                                         start=False, stop=(h == HEADS - 1))
                    ot = outp.tile([128, 512], F32, tag="ot")
                    nc.vector.tensor_tensor(out=ot[:, 0:qn], in0=pp[:, 0:qn],
                                            in1=xq_sb[:, mc, q0:q0 + qn], op=ALU.add)
                    nc.sync.dma_start(out=out_d[128 * mc:128 * (mc + 1), q0:q0 + qn], in_=ot[:, 0:qn])
            return fn

        # sweep A (hp=0): epilogue of window w-1 injected before window w's
        # first PV (o_t reuse WAR requires it) — use inject key -1 (pre-loop).
        eA0 = make_epilogue(0, 0, run_attention(0, 0))
        eA1 = make_epilogue(0, 1, run_attention(0, 1, {-1: [eA0]}))
        eA2 = make_epilogue(0, 2, run_attention(0, 2, {-1: [eA1]}))
        # sweep B (hp=1): also interleave sweep-A tail epilogue and projections
        eB0 = make_epilogue(1, 0, run_attention(1, 0, {-1: [eA2]}))
        eB1 = make_epilogue(1, 1, run_attention(1, 1, {-1: [eB0], 3: [make_proj(0)]}))
        eB2 = make_epilogue(1, 2, run_attention(1, 2, {-1: [eB1], 3: [make_proj(1)]}))
        eB2()
        make_proj(2)()


def make_inputs(x, norm_w, norm_b, qkv_w, qkv_b, proj_w, proj_b):
    """Host-side prep: full-input numpy -> per-core in_maps."""
    x2 = np.ascontiguousarray(np.asarray(x, np.float32).reshape(C, SEQ))
    qkv_w = np.asarray(qkv_w, np.float32)
    qkv_b = np.asarray(qkv_b, np.float32)
    proj_w = np.asarray(proj_w, np.float32)
    proj_b = np.asarray(proj_b, np.float32)
    norm_w = np.asarray(norm_w, np.float32)
    norm_b = np.asarray(norm_b, np.float32)

    wT = np.ascontiguousarray(qkv_w.T)
    projT = np.ascontiguousarray(proj_w.T)  # [c_in, c_out]
    projTh = np.ascontiguousarray(
        projT.reshape(4, 64, C).transpose(1, 0, 2).reshape(64, 4 * C))
    # v-bias folds into the projection bias: proj(attn + bv) = proj(attn) + proj_w @ bv
    projbr = (proj_b + proj_w @ qkv_b[2 * C:3 * C]).reshape(1, C).astype(np.float32)
    pvec = np.stack([
        qkv_b[0:128], qkv_b[128:256],            # q bias bank 0/1
        qkv_b[C:C + 128], qkv_b[C + 128:2 * C],  # k bias bank 0/1
        norm_w[0:128], norm_w[128:256],
        norm_b[0:128], norm_b[128:256],
    ], axis=1).astype(np.float32)
    cidx = np.arange(128)
    gidx = np.arange(16)
    G = ((cidx[:, None] // 8) == gidx[None, :]).astype(np.float32) / 8.0
    GT = np.ascontiguousarray(G.T * 8.0)

    common = dict(x=x2, wT=wT, projTh=projTh, pvec=pvec, projbr=projbr, G=G, GT=GT)
    in_maps = []
    cols = []
    for i in range(NCORES):
        ci = np.concatenate([
            np.arange(512 * i, 512 * (i + 1)),
            np.arange(4096 + 512 * i, 4096 + 512 * (i + 1)),
            np.arange(8192 + 128 * i, 8192 + 128 * (i + 1)),
        ])
        cols.append(ci)
        m = dict(common)
        m["xq"] = np.ascontiguousarray(x2[:, ci])
        in_maps.append(m)
    return in_maps, cols


_NC_CACHE = {}


def kernel(x, norm_w, norm_b, qkv_w, qkv_b, proj_w, proj_b):
    from concourse.bass_utils import run_bass_kernel_spmd

    _patch_tile_drain()
    _patch_to_json_split_waits()
    in_maps, cols = make_inputs(x, norm_w, norm_b, qkv_w, qkv_b, proj_w, proj_b)
    if "nc" not in _NC_CACHE:
        _NC_CACHE["nc"] = build_nc()
    nc = _NC_CACHE["nc"]
    res = run_bass_kernel_spmd(nc, in_maps, core_ids=list(range(NCORES)))
    out = np.zeros((C, SEQ), np.float32)
    for i in range(NCORES):
        out[:, cols[i]] = res.results[i]["out"]
    return out.reshape(1, C, 96, 96)


# revision 16
# speedup vs baseline: 1.4526x; 1.0792x over previous
"""Trainium2 Bass kernel for nn_AttentionBlock (GroupNorm + windowed MHA + proj + residual).

Contract: kernel(**inputs) takes FULL unsharded inputs (as from reference.setup_inputs())
and returns the FULL output [1, 256, 96, 96] float32.

Sharding: sequence-parallel over query positions across 8 cores. Each core gets a
uniform slice of each of the 3 reference attention windows:
  W0: q[512i   : 512(i+1)]    attends kv[0    : 6144]
  W1: q[4096+512i : ...]      attends kv[2048 : 9216]
  W2: q[8192+128i : ...]      attends kv[6144 : 9216]
All 4 heads for those queries are computed locally, so the output projection and
residual are local too. Every core redundantly computes GroupNorm stats and the
full-sequence K/V (its key windows span the whole sequence).

Phase structure (single core):
  1. stats: x [256,9216] f32 streamed in on 4 DMA queues; bn_stats on DVE;
     ScalarE copies each tile to a resident fp16 buffer (xn).
  2. GroupNorm reduce -> per-channel a,b (tiny PE matmuls against group maps).
  3. xn normalized IN PLACE on DVE (fp16, 2x/4x DVE modes); xq -> xnq fp16.
  4. qkv for ALL 4 heads once: K^T [2 head-pairs x 128, seq] fp16, V [token-chunk,
     head, 64|1] fp16 (65th col ones -> softmax sums fall out of the PV matmul),
     q per window fp16. All matmul operands fp16 (PE streams 2-byte lhsT faster;
     fp32 PSUM accumulation keeps precision; rel err ~1e-3 << 2e-2 gate).
  5. Attention sweep per head-pair hp: for each window, stream key-chunks in
     SCHUNK=2 groups: QK pair (2 heads interleaved on PE row-tiles 0/64 -> they
     run concurrently), exp on ScalarE (PSUM->SBUF fp16), PV pair accumulating
     O^T[65, qn] per head in PSUM. QK is emitted ONE GROUP AHEAD of exp/PV:
     the PE executes in order, so group g's PV (gated on exp g) must sit after
     group g+1's QK or the PE idles every group and HAM-downclocks.
     Epilogue per window: sums row -> PE ones-broadcast -> DVE reciprocal+mult
     -> attn [64, head, q] fp16 (everything stays on partitions 0:64 so no
     partition-moving DMAs). Epilogues/projections are injected into the next
     window's group loop so they overlap attention instead of serializing.
  6. Projection per window (after sweep 1): K=64 matmuls per head against a
     head-major projTh layout + rank-1 bias matmul + residual, DMA out.
"""

import numpy as np

import concourse.bass as bass
import concourse.tile as tile
from concourse import mybir
from concourse.vector_clock import ScopedClock, VectorClock

F32 = mybir.dt.float32
F32R = mybir.dt.float32r
F16 = mybir.dt.float16
AF = mybir.ActivationFunctionType
ALU = mybir.AluOpType

C = 256
SEQ = 9216
NCORES = 8
HEADS = 4
D = 64
EPS = 1e-5
SCALE = 0.125  # 1/sqrt(64)
NQC = 1152  # queries per core
ST = 512  # seq tile for qkv
NST = SEQ // ST  # 18
NCH = SEQ // 128  # 72 key chunks
# windows: (q_off, q_len, key_chunk0, n_key_chunks)
WINDOWS = [(0, 512, 0, 48), (512, 512, 16, 56), (1024, 128, 48, 24)]
SCHUNK = 2  # key-chunk items batched per exp ACTIVATE (2 PSUM banks)


def _patch_tile_drain():
    """This container's walrus rejects >1 sem wait on one sync CTRL instruction
    ("Too many sync wait commands"). Split the TileContext-exit drain's waits
    into one-wait-per-nop instructions."""
    if getattr(tile.TileContext, "_drain_split_patched", False):
        return

    def _drain_and_barrier(self, tick_clock, wait_clock):
        vc = tick_clock.global_clock
        n = len(vc)
        for p in range(n):
            t = vc[p]
            if t <= 0:
                continue
            single = VectorClock([t if i == p else 0 for i in range(n)])
            inst = self.nc.sync.nop(nofuse=True, hint="drain_split_wait")
            wait_clock.add_sem_waits(inst.ins, ScopedClock({None: single}))
        self.nc.sync.drain()
        self.nc.all_engine_barrier()
        assert self.sems is not None
        popped = self.nc._tile_sem_poison_stack.pop()
        assert popped is self._sem_poison
        self.nc.clear_and_free_semaphores(list(self.sems.allocated().values()))
        self.nc.all_engine_barrier()

    tile.TileContext._drain_and_barrier = _drain_and_barrier
    tile.TileContext._drain_split_patched = True


def _patch_to_json_split_waits():
    """This walrus build rejects instructions carrying more than one sem-wait
    ("Too many sync wait commands"). Post-process the BIR JSON: keep one wait
    on each instruction and move extras onto same-engine NoOps inserted just
    before it (identical sync semantics: the engine blocks on the nops first)."""
    if getattr(bass.Bass, "_split_waits_patched", False):
        return
    import json as _json

    orig = bass.Bass.to_json_bytes

    def to_json_bytes(self):
        d = _json.loads(orig(self))
        for fn in d["functions"]:
            for blk in fn["blocks"]:
                out = []
                changed = False
                for ins in blk["instructions"]:
                    si = ins.get("sync_info")
                    waits = (si or {}).get("on_wait") or []
                    if len(waits) > 1:
                        changed = True
                        for k, w in enumerate(waits[:-1]):
                            out.append({
                                "debug": ins.get("debug", 0),
                                "engine": ins["engine"],
                                "ins": [],
                                "name": f"{ins['name']}-w{k}",
                                "opcode": "NoOp",
                                "outs": [],
                                "sync_info": {"on_update": [], "on_wait": [w]},
                                "text_hint": "split_wait",
                            })
                        si["on_wait"] = [waits[-1]]
                    out.append(ins)
                if changed:
                    blk["instructions"] = out
        return _json.dumps(d).encode()

    bass.Bass.to_json_bytes = to_json_bytes
    bass.Bass._split_waits_patched = True


def declare_drams(nc):
    d = {}
    d["x"] = nc.dram_tensor("x", [C, SEQ], F16, kind="ExternalInput")
    d["xq"] = nc.dram_tensor("xq", [C, NQC], F32, kind="ExternalInput")
    d["wT"] = nc.dram_tensor("wT", [C, 3 * C], F32, kind="ExternalInput")
    d["projTh"] = nc.dram_tensor("projTh", [64, 4 * C], F32, kind="ExternalInput")
    d["pvec"] = nc.dram_tensor("pvec", [128, 8], F32, kind="ExternalInput")
    d["projbr"] = nc.dram_tensor("projbr", [1, C], F32, kind="ExternalInput")
    d["G"] = nc.dram_tensor("G", [128, 16], F32, kind="ExternalInput")
    d["GT"] = nc.dram_tensor("GT", [16, 128], F32, kind="ExternalInput")
    d["out"] = nc.dram_tensor("out", [C, NQC], F32, kind="ExternalOutput")
    return d


def build_nc(reps=1):
    """reps>1 re-emits the whole kernel body back-to-back inside one NEFF —
    used only for timing (amortizes the ~2.5ms axon dispatch cost per
    execution; device time per iteration = slope between two reps values)."""
    nc = bass.Bass()
    d = declare_drams(nc)
    with tile.TileContext(nc) as tc:
        for _rep in range(reps):
            _build_body(nc, tc, d)
    return nc


def _build_body(nc, tc, d):
    x_d, xq_d, wT_d, projTh_d = d["x"], d["xq"], d["wT"], d["projTh"]
    pvec_d, projbr_d, G_d, GT_d, out_d = d["pvec"], d["projbr"], d["G"], d["GT"], d["out"]

    with (
        tc.tile_pool(name="singles", bufs=1) as singles,
        tc.tile_pool(name="pt", bufs=2) as ptp,
        tc.tile_pool(name="epi", bufs=2) as epi,
        tc.tile_pool(name="outp", bufs=2) as outp,
        tc.tile_pool(name="pg", bufs=4) as pg,
        tc.tile_pool(name="sps", bufs=2, space="PSUM") as sps,
        tc.tile_pool(name="accps", bufs=2, space="PSUM") as accps,
        tc.tile_pool(name="ops", bufs=1, space="PSUM") as ops,
    ):
        # ---- constants: DMA + fp16 conversion (staging pool freed after) ----
        pvec_sb = singles.tile([128, 8], F32, tag="pvec")
        nc.sync.dma_start(out=pvec_sb, in_=pvec_d[:, :])
        G_sb = singles.tile([128, 16], F32, tag="G")
        nc.sync.dma_start(out=G_sb, in_=G_d[:, :])
        GT_sb = singles.tile([16, 128], F32, tag="GT")
        nc.sync.dma_start(out=GT_sb, in_=GT_d[:, :])
        xq_sb = singles.tile([128, 2, NQC], F32, tag="xq")
        nc.sync.dma_start(out=xq_sb[:, 0, :], in_=xq_d[0:128, :])
        nc.sync.dma_start(out=xq_sb[:, 1, :], in_=xq_d[128:256, :])

        wT_h = singles.tile([128, 2, 3 * C], F16, tag="wT_h")
        projTh_h = singles.tile([64, 4, C], F16, tag="projTh_h")
        projbr_h = singles.tile([1, C], F16, tag="projbr_h")
        with tc.tile_pool(name="wstage", bufs=1) as wstage:
            ws = wstage.tile([128, 2, 3 * C], F32, tag="ws")
            nc.sync.dma_start(out=ws[:, 0, :], in_=wT_d[0:128, :])
            nc.sync.dma_start(out=ws[:, 1, :], in_=wT_d[128:256, :])
            nc.vector.tensor_copy(out=wT_h[:, 0, :], in_=ws[:, 0, :])
            nc.vector.tensor_copy(out=wT_h[:, 1, :], in_=ws[:, 1, :])
            ps = wstage.tile([64, 4 * C], F32, tag="ps")
            nc.sync.dma_start(out=ps, in_=projTh_d[:, :])
            nc.vector.tensor_copy(out=projTh_h.rearrange("p h c -> p (h c)"), in_=ps)
            pbs = wstage.tile([1, C], F32, tag="pbs")
            nc.sync.dma_start(out=pbs, in_=projbr_d[:, :])
            nc.vector.tensor_copy(out=projbr_h, in_=pbs)

        ones_h = singles.tile([1, 512], F16, tag="ones_h")
        nc.vector.memset(ones_h, 1.0)
        ones_f32 = singles.tile([128, 64], F32, tag="ones_f32")
        nc.vector.memset(ones_f32, 1.0)
        ones_c = singles.tile([128, 1], F16, tag="ones_c")
        nc.vector.memset(ones_c, 1.0)

        # ---- persistent fp16 state ----
        xn = singles.tile([128, 2, SEQ], F16, tag="xn")          # raw x, then normalized
        k_res = singles.tile([128, 2, SEQ], F16, tag="k_res")    # [2-head rows, hp, token]
        v_res = singles.tile([128, NCH, 2, 130], F16, tag="v_res")  # [tok%128, chunk, hp, 2x(64|1)]
        q_res = singles.tile([128, 2, 3, 512], F16, tag="q_res")  # [2-head rows, hp, window, q]
        attn_sb = singles.tile([64, HEADS, NQC], F16, tag="attn")  # [dim, head, q]
        xnq = singles.tile([128, 2, NQC], F16, tag="xnq")

        # ones columns of v (col 64 of each head slot) — written once
        v5 = v_res.rearrange("p ch hp (hl c) -> p ch hp hl c", hl=2)
        ones_bc = bass.AP(tensor=ones_c.tensor, offset=ones_c.offset,
                          ap=[list(ones_c.ap[0]), [0, NCH], [0, 2], [0, 2], [1, 1]])
        nc.vector.tensor_copy(out=v5[:, :, :, :, 64:65], in_=ones_bc)

        # ---- phase 1: x (fp16, host-cast) streamed straight into xn on 3
        # DMA queues + bn_stats ----
        stats = singles.tile([128, 2, NST, 6], F32, tag="stats")
        qeng = [nc.gpsimd, nc.scalar, nc.sync]
        for bt in range(NST // 2):  # 9 big tiles of [128, 1024] per cc
            for cc in range(2):
                sl = slice(2 * ST * bt, 2 * ST * (bt + 1))
                qeng[(2 * bt + cc) % 3].dma_start(
                    out=xn[:, cc, sl], in_=x_d[128 * cc:128 * (cc + 1), sl])
                nc.vector.bn_stats(out=stats[:, cc, 2 * bt, :],
                                   in_=xn[:, cc, 2 * ST * bt:2 * ST * bt + ST])
                nc.vector.bn_stats(out=stats[:, cc, 2 * bt + 1, :],
                                   in_=xn[:, cc, 2 * ST * bt + ST:2 * ST * (bt + 1)])

        # ---- phase 2: GroupNorm stats -> per-channel a, b ----
        ab_sb = singles.tile([128, 2, 2], F32, tag="ab")  # [:, cc, 0]=a, [:, cc, 1]=b
        for cc in range(2):
            mv = pg.tile([128, 2], F32, tag="mv")
            nc.vector.bn_aggr(out=mv, in_=stats[:, cc, :, :])
            st2 = pg.tile([128, 2], F32, tag="st2")  # (mean, E[x^2])
            nc.vector.tensor_copy(out=st2[:, 0:1], in_=mv[:, 0:1])
            nc.vector.tensor_tensor(out=st2[:, 1:2], in0=mv[:, 0:1], in1=mv[:, 0:1], op=ALU.mult)
            nc.vector.tensor_tensor(out=st2[:, 1:2], in0=st2[:, 1:2], in1=mv[:, 1:2], op=ALU.add)
            gps = accps.tile([128, 512], F32, tag="acc")
            nc.tensor.matmul(gps[0:16, 0:2], lhsT=G_sb, rhs=st2, start=True, stop=True)
            gm = pg.tile([16, 2], F32, tag="gm")  # (mean_g, E2_g)
            nc.vector.tensor_copy(out=gm, in_=gps[0:16, 0:2])
            t16 = pg.tile([16, 1], F32, tag="t16")
            nc.vector.tensor_tensor(out=t16, in0=gm[:, 0:1], in1=gm[:, 0:1], op=ALU.mult)
            nc.vector.tensor_tensor(out=gm[:, 1:2], in0=gm[:, 1:2], in1=t16, op=ALU.subtract)
            # rstd = 1/sqrt(var+eps)
            nc.vector.tensor_scalar_add(out=gm[:, 1:2], in0=gm[:, 1:2], scalar1=EPS)
            nc.scalar.activation(out=gm[:, 1:2], in_=gm[:, 1:2], func=AF.Sqrt)
            nc.vector.reciprocal(out=gm[:, 1:2], in_=gm[:, 1:2])
            mps = accps.tile([128, 512], F32, tag="acc")
            nc.tensor.matmul(mps[0:128, 0:2], lhsT=GT_sb, rhs=gm, start=True, stop=True)
            mr = pg.tile([128, 2], F32, tag="mr")  # (mean_c, rstd_c)
            nc.vector.tensor_copy(out=mr, in_=mps[0:128, 0:2])
            # a = rstd * norm_w ; b = norm_b - mean * a
            nc.vector.tensor_tensor(out=ab_sb[:, cc, 0:1], in0=mr[:, 1:2], in1=pvec_sb[:, 4 + cc:5 + cc], op=ALU.mult)
            t128 = pg.tile([128, 1], F32, tag="t128")
            nc.vector.tensor_tensor(out=t128, in0=mr[:, 0:1], in1=ab_sb[:, cc, 0:1], op=ALU.mult)
            nc.vector.tensor_tensor(out=ab_sb[:, cc, 1:2], in0=pvec_sb[:, 6 + cc:7 + cc], in1=t128, op=ALU.subtract)

        # ---- phase 3: xq -> xnq (xn normalized per-tile inside the qkv loop) ----
        for cc in range(2):
            nc.vector.tensor_scalar(
                out=xnq[:, cc, :], in0=xq_sb[:, cc, :],
                scalar1=ab_sb[:, cc, 0:1], scalar2=ab_sb[:, cc, 1:2],
                op0=ALU.mult, op1=ALU.add)

        # ---- phase 4: qkv for all 4 heads ----
        def emit_qkv_tile(st):
            s0 = ST * st
            for cc in range(2):  # normalize this tile's xn slice in place (fp16 2x DVE)
                nc.vector.tensor_scalar(
                    out=xn[:, cc, s0:s0 + ST], in0=xn[:, cc, s0:s0 + ST],
                    scalar1=ab_sb[:, cc, 0:1], scalar2=ab_sb[:, cc, 1:2],
                    op0=ALU.mult, op1=ALU.add)
            for kb in range(2):  # k rows [128kb:128kb+128] = head-pair kb
                kps = accps.tile([128, 512], F32, tag="acc", name="kps")
                for cc in range(2):
                    nc.tensor.matmul(
                        kps, lhsT=wT_h[:, cc, C + 128 * kb:C + 128 * kb + 128],
                        rhs=xn[:, cc, s0:s0 + ST], start=(cc == 0), stop=(cc == 1))
                nc.vector.tensor_scalar_add(
                    out=k_res[:, kb, s0:s0 + ST], in0=kps,
                    scalar1=pvec_sb[:, 2 + kb:3 + kb])
            for mc in range(4):  # token sub-chunks of 128
                vps = accps.tile([128, 512], F32, tag="acc", name="vps")
                for cc in range(2):
                    nc.tensor.matmul(
                        vps[:, 0:256],
                        lhsT=xn[:, cc, s0 + 128 * mc:s0 + 128 * (mc + 1)],
                        rhs=wT_h[:, cc, 2 * C:3 * C],
                        start=(cc == 0), stop=(cc == 1))
                ch = 4 * st + mc
                vpsv = vps[:, 0:256].rearrange("p (hp hl c) -> p hp hl c", hp=2, hl=2)
                nc.vector.tensor_copy(out=v5[:, ch, :, :, 0:64], in_=vpsv)

        def emit_q():
            for w in range(3):
                qn = WINDOWS[w][1]
                for kb in range(2):
                    qps = accps.tile([128, 512], F32, tag="acc", name="qps")
                    for cc in range(2):
                        nc.tensor.matmul(
                            qps[:, 0:qn], lhsT=wT_h[:, cc, 128 * kb:128 * kb + 128],
                            rhs=xnq[:, cc, 512 * w:512 * w + qn], start=(cc == 0), stop=(cc == 1))
                    nc.vector.tensor_scalar_add(out=q_res[:, kb, w, 0:qn],
                                                in0=qps[:, 0:qn], scalar1=pvec_sb[:, kb:kb + 1])

        # ---- attention ----
        o_tiles = {}

        def gen_attention(hp, w):
            """Generator: one yield per S-tile group (for interleaved emission)."""
            q0, qn, kc0, nch = WINDOWS[w]
            o_t = {hl: ops.tile([128, 512], F32, tag=f"o{hl}", name=f"o{hl}") for hl in range(2)}
            o_tiles[(hp, w)] = o_t
            stream = [(hl, kc0 + c) for c in range(nch) for hl in range(2)]
            groups = [stream[i:i + SCHUNK] for i in range(0, len(stream), SCHUNK)]
            s_tiles = {}

            def emit_qk(g):
                # each QK matmul output must start on a PSUM bank boundary
                s_ps = sps.tile([128, 2, 512], F32, tag="s", name="s_ps")
                for j, (hl, kc) in enumerate(groups[g]):
                    nc.tensor.matmul(
                        s_ps[:, j, 0:qn],
                        lhsT=k_res[64 * hl:64 * hl + 64, hp, 128 * kc:128 * kc + 128],
                        rhs=q_res[64 * hl:64 * hl + 64, hp, w, 0:qn],
                        start=True, stop=True)
                s_tiles[g] = s_ps

            emit_qk(0)
            for g in range(len(groups)):
                if g + 1 < len(groups):
                    emit_qk(g + 1)
                items = groups[g]
                m = len(items)
                s_ps = s_tiles.pop(g)
                pt = ptp.tile([128, 2 * 512], F16, tag="p", name="pt")
                ptv = pt[:, 0:m * qn].rearrange("p (j c) -> p j c", j=m)
                nc.scalar.activation(out=ptv, in_=s_ps[:, 0:m, 0:qn], func=AF.Exp, scale=SCALE)
                for j, (hl, kc) in enumerate(items):
                    nc.tensor.matmul(
                        o_t[hl][0:65, 0:qn],
                        lhsT=v5[:, kc, hp, hl, :],
                        rhs=pt[:, qn * j:qn * (j + 1)],
                        start=(kc == kc0), stop=(kc == kc0 + nch - 1))
                yield

        def drive(gen, inject=None):
            """Consume gen; inject[-1] fns emit before it starts, inject[g]
            right after group g (epilogues/projections of earlier windows, so
            they overlap this window's attention)."""
            inject = inject or {}
            for fn in inject.get(-1, []):
                fn()
            g = 0
            for _ in gen:
                for fn in inject.get(g, []):
                    fn()
                g += 1

        def make_epilogue(hp, w):
            """O^T[0:64] / O^T[64] -> attn[0:64, head, q]  (all on partitions 0:64)."""
            q0, qn, _, _ = WINDOWS[w]
            o_t = o_tiles[(hp, w)]

            def fn():
                for hl in range(2):
                    osb = epi.tile([65, 512], F32, tag="osb", name="osb")
                    nc.vector.tensor_copy(out=osb[:, 0:qn], in_=o_t[hl][0:65, 0:qn])
                    # sums row -> broadcast [64, qn] on PE (fp32 rank-1), then
                    # reciprocal+mult on DVE across all 64 lanes at once
                    bps = accps.tile([128, 512], F32, tag="acc", name="bps")
                    nc.tensor.matmul(bps[0:64, 0:qn], lhsT=ones_f32[64:65, :],
                                     rhs=osb[64:65, 0:qn], start=True, stop=True)
                    recb = epi.tile([64, 512], F32, tag="recb", name="recb")
                    nc.vector.reciprocal(out=recb[:, 0:qn], in_=bps[0:64, 0:qn])
                    nc.vector.tensor_tensor(
                        out=attn_sb[:, 2 * hp + hl, q0:q0 + qn],
                        in0=osb[0:64, 0:qn], in1=recb[:, 0:qn], op=ALU.mult)
            return fn

        def make_proj(w):
            q0, qn, _, _ = WINDOWS[w]

            def fn():
                for mc in range(2):
                    pp = accps.tile([128, 512], F32, tag="acc", name="pp")
                    nc.tensor.matmul(pp[:, 0:qn], lhsT=projbr_h[0:1, 128 * mc:128 * (mc + 1)],
                                     rhs=ones_h[0:1, 0:qn], start=True, stop=False)
                    for h in range(HEADS):
                        nc.tensor.matmul(pp[:, 0:qn], lhsT=projTh_h[:, h, 128 * mc:128 * (mc + 1)],
                                         rhs=attn_sb[:, h, q0:q0 + qn],
                                         start=False, stop=(h == HEADS - 1))
                    ot = outp.tile([128, 512], F32, tag="ot")
                    nc.vector.tensor_tensor(out=ot[:, 0:qn], in0=pp[:, 0:qn],
                                            in1=xq_sb[:, mc, q0:q0 + qn], op=ALU.add)
                    nc.sync.dma_start(out=out_d[128 * mc:128 * (mc + 1), q0:q0 + qn], in_=ot[:, 0:qn])
            return fn

        # qkv emission interleaved with sweep-A W0 attention: pump a W0 group
        # as soon as the key-chunks its (one-ahead) QK reads are written.
        emit_q()
        gA0 = gen_attention(0, 0)
        NG0 = (2 * WINDOWS[0][3] + SCHUNK - 1) // SCHUNK
        pumped = 0
        for st in range(NST):
            emit_qkv_tile(st)
            while pumped < NG0 and (SCHUNK * (pumped + 2) - 1) // 2 <= 4 * st + 3:
                next(gA0, None)
                pumped += 1
        for _ in gA0:
            pass
        # remaining sweep A, then sweep B; window w-1's epilogue is injected
        # before window w's first PV (o_t bank reuse needs it emitted first),
        # projections injected a few groups in.
        drive(gen_attention(0, 1), {-1: [make_epilogue(0, 0)]})
        drive(gen_attention(0, 2), {-1: [make_epilogue(0, 1)]})
        drive(gen_attention(1, 0), {-1: [make_epilogue(0, 2)]})
        drive(gen_attention(1, 1), {-1: [make_epilogue(1, 0)], 3: [make_proj(0)]})
        drive(gen_attention(1, 2), {-1: [make_epilogue(1, 1)], 3: [make_proj(1)]})
        make_epilogue(1, 2)()
        make_proj(2)()


def make_inputs(x, norm_w, norm_b, qkv_w, qkv_b, proj_w, proj_b):
    """Host-side prep: full-input numpy -> per-core in_maps."""
    x2 = np.ascontiguousarray(np.asarray(x, np.float32).reshape(C, SEQ))
    x16 = x2.astype(np.float16)
    qkv_w = np.asarray(qkv_w, np.float32)
    qkv_b = np.asarray(qkv_b, np.float32)
    proj_w = np.asarray(proj_w, np.float32)
    proj_b = np.asarray(proj_b, np.float32)
    norm_w = np.asarray(norm_w, np.float32)
    norm_b = np.asarray(norm_b, np.float32)

    wT = np.ascontiguousarray(qkv_w.T)
    projT = np.ascontiguousarray(proj_w.T)  # [c_in, c_out]
    projTh = np.ascontiguousarray(
        projT.reshape(4, 64, C).transpose(1, 0, 2).reshape(64, 4 * C))
    # v-bias folds into the projection bias: proj(attn + bv) = proj(attn) + proj_w @ bv
    projbr = (proj_b + proj_w @ qkv_b[2 * C:3 * C]).reshape(1, C).astype(np.float32)
    pvec = np.stack([
        qkv_b[0:128], qkv_b[128:256],            # q bias bank 0/1
        qkv_b[C:C + 128], qkv_b[C + 128:2 * C],  # k bias bank 0/1
        norm_w[0:128], norm_w[128:256],
        norm_b[0:128], norm_b[128:256],
    ], axis=1).astype(np.float32)
    cidx = np.arange(128)
    gidx = np.arange(16)
    G = ((cidx[:, None] // 8) == gidx[None, :]).astype(np.float32) / 8.0
    GT = np.ascontiguousarray(G.T * 8.0)

    common = dict(x=x16, wT=wT, projTh=projTh, pvec=pvec, projbr=projbr, G=G, GT=GT)
    in_maps = []
    cols = []
    for i in range(NCORES):
        ci = np.concatenate([
            np.arange(512 * i, 512 * (i + 1)),
            np.arange(4096 + 512 * i, 4096 + 512 * (i + 1)),
            np.arange(8192 + 128 * i, 8192 + 128 * (i + 1)),
        ])
        cols.append(ci)
        m = dict(common)
        m["xq"] = np.ascontiguousarray(x2[:, ci])
        in_maps.append(m)
    return in_maps, cols


_NC_CACHE = {}


def kernel(x, norm_w, norm_b, qkv_w, qkv_b, proj_w, proj_b):
    from concourse.bass_utils import run_bass_kernel_spmd

    _patch_tile_drain()
    _patch_to_json_split_waits()
    in_maps, cols = make_inputs(x, norm_w, norm_b, qkv_w, qkv_b, proj_w, proj_b)
    if "nc" not in _NC_CACHE:
        _NC_CACHE["nc"] = build_nc()
    nc = _NC_CACHE["nc"]
    res = run_bass_kernel_spmd(nc, in_maps, core_ids=list(range(NCORES)))
    out = np.zeros((C, SEQ), np.float32)
    for i in range(NCORES):
        out[:, cols[i]] = res.results[i]["out"]
    return out.reshape(1, C, 96, 96)


# revision 20
# speedup vs baseline: 1.4546x; 1.0014x over previous
"""Trainium2 Bass kernel for nn_AttentionBlock (GroupNorm + windowed MHA + proj + residual).

Contract: kernel(**inputs) takes FULL unsharded inputs (as from reference.setup_inputs())
and returns the FULL output [1, 256, 96, 96] float32.

Sharding: sequence-parallel over query positions across 8 cores. Each core gets a
uniform slice of each of the 3 reference attention windows:
  W0: q[512i   : 512(i+1)]    attends kv[0    : 6144]
  W1: q[4096+512i : ...]      attends kv[2048 : 9216]
  W2: q[8192+128i : ...]      attends kv[6144 : 9216]
All 4 heads for those queries are computed locally, so the output projection and
residual are local too. Every core redundantly computes GroupNorm stats and the
full-sequence K/V (its key windows span the whole sequence).

Phase structure (single core):
  1. stats: x [256,9216] f32 streamed in on 4 DMA queues; bn_stats on DVE;
     ScalarE copies each tile to a resident fp16 buffer (xn).
  2. GroupNorm reduce -> per-channel a,b (tiny PE matmuls against group maps).
  3. xn normalized IN PLACE on DVE (fp16, 2x/4x DVE modes); xq -> xnq fp16.
  4. qkv for ALL 4 heads once: K^T [2 head-pairs x 128, seq] fp16, V [token-chunk,
     head, 64|1] fp16 (65th col ones -> softmax sums fall out of the PV matmul),
     q per window fp16. All matmul operands fp16 (PE streams 2-byte lhsT faster;
     fp32 PSUM accumulation keeps precision; rel err ~1e-3 << 2e-2 gate).
  5. Attention sweep per head-pair hp: for each window, stream key-chunks in
     SCHUNK=2 groups: QK pair (2 heads interleaved on PE row-tiles 0/64 -> they
     run concurrently), exp on ScalarE (PSUM->SBUF fp16), PV pair accumulating
     O^T[65, qn] per head in PSUM. QK is emitted ONE GROUP AHEAD of exp/PV:
     the PE executes in order, so group g's PV (gated on exp g) must sit after
     group g+1's QK or the PE idles every group and HAM-downclocks.
     Epilogue per window: sums row -> PE ones-broadcast -> DVE reciprocal+mult
     -> attn [64, head, q] fp16 (everything stays on partitions 0:64 so no
     partition-moving DMAs). Epilogues/projections are injected into the next
     window's group loop so they overlap attention instead of serializing.
  6. Projection per window (after sweep 1): K=64 matmuls per head against a
     head-major projTh layout + rank-1 bias matmul + residual, DMA out.
"""

import numpy as np

import concourse.bass as bass
import concourse.tile as tile
from concourse import mybir
from concourse.vector_clock import ScopedClock, VectorClock

F32 = mybir.dt.float32
F32R = mybir.dt.float32r
F16 = mybir.dt.float16
AF = mybir.ActivationFunctionType
ALU = mybir.AluOpType

C = 256
SEQ = 9216
NCORES = 8
HEADS = 4
D = 64
EPS = 1e-5
SCALE = 0.125  # 1/sqrt(64)
NQC = 1152  # queries per core
ST = 512  # seq tile for qkv
NST = SEQ // ST  # 18
NCH = SEQ // 128  # 72 key chunks
# windows: (q_off, q_len, key_chunk0, n_key_chunks)
WINDOWS = [(0, 512, 0, 48), (512, 512, 16, 56), (1024, 128, 48, 24)]
SCHUNK = 2  # key-chunk items batched per exp ACTIVATE (2 PSUM banks)


def _patch_tile_drain():
    """This container's walrus rejects >1 sem wait on one sync CTRL instruction
    ("Too many sync wait commands"). Split the TileContext-exit drain's waits
    into one-wait-per-nop instructions."""
    if getattr(tile.TileContext, "_drain_split_patched", False):
        return

    def _drain_and_barrier(self, tick_clock, wait_clock):
        vc = tick_clock.global_clock
        n = len(vc)
        for p in range(n):
            t = vc[p]
            if t <= 0:
                continue
            single = VectorClock([t if i == p else 0 for i in range(n)])
            inst = self.nc.sync.nop(nofuse=True, hint="drain_split_wait")
            wait_clock.add_sem_waits(inst.ins, ScopedClock({None: single}))
        self.nc.sync.drain()
        self.nc.all_engine_barrier()
        assert self.sems is not None
        popped = self.nc._tile_sem_poison_stack.pop()
        assert popped is self._sem_poison
        self.nc.clear_and_free_semaphores(list(self.sems.allocated().values()))
        self.nc.all_engine_barrier()

    tile.TileContext._drain_and_barrier = _drain_and_barrier
    tile.TileContext._drain_split_patched = True


def _patch_to_json_split_waits():
    """This walrus build rejects instructions carrying more than one sem-wait
    ("Too many sync wait commands"). Post-process the BIR JSON: keep one wait
    on each instruction and move extras onto same-engine NoOps inserted just
    before it (identical sync semantics: the engine blocks on the nops first)."""
    if getattr(bass.Bass, "_split_waits_patched", False):
        return
    import json as _json

    orig = bass.Bass.to_json_bytes

    def to_json_bytes(self):
        d = _json.loads(orig(self))
        for fn in d["functions"]:
            for blk in fn["blocks"]:
                out = []
                changed = False
                for ins in blk["instructions"]:
                    si = ins.get("sync_info")
                    waits = (si or {}).get("on_wait") or []
                    if len(waits) > 1:
                        changed = True
                        for k, w in enumerate(waits[:-1]):
                            out.append({
                                "debug": ins.get("debug", 0),
                                "engine": ins["engine"],
                                "ins": [],
                                "name": f"{ins['name']}-w{k}",
                                "opcode": "NoOp",
                                "outs": [],
                                "sync_info": {"on_update": [], "on_wait": [w]},
                                "text_hint": "split_wait",
                            })
                        si["on_wait"] = [waits[-1]]
                    out.append(ins)
                if changed:
                    blk["instructions"] = out
        return _json.dumps(d).encode()

    bass.Bass.to_json_bytes = to_json_bytes
    bass.Bass._split_waits_patched = True


def declare_drams(nc):
    d = {}
    d["x"] = nc.dram_tensor("x", [C, SEQ], F16, kind="ExternalInput")
    d["xq"] = nc.dram_tensor("xq", [C, NQC], F32, kind="ExternalInput")
    d["wT"] = nc.dram_tensor("wT", [C, 3 * C], F32, kind="ExternalInput")
    d["projTh"] = nc.dram_tensor("projTh", [64, 4 * C], F32, kind="ExternalInput")
    d["pvec"] = nc.dram_tensor("pvec", [128, 8], F32, kind="ExternalInput")
    d["projbr"] = nc.dram_tensor("projbr", [1, C], F32, kind="ExternalInput")
    d["G"] = nc.dram_tensor("G", [128, 16], F32, kind="ExternalInput")
    d["GT"] = nc.dram_tensor("GT", [16, 128], F32, kind="ExternalInput")
    d["out"] = nc.dram_tensor("out", [C, NQC], F32, kind="ExternalOutput")
    return d


def build_nc(reps=1):
    """reps>1 re-emits the whole kernel body back-to-back inside one NEFF —
    used only for timing (amortizes the ~2.5ms axon dispatch cost per
    execution; device time per iteration = slope between two reps values)."""
    nc = bass.Bass()
    d = declare_drams(nc)
    with tile.TileContext(nc) as tc:
        for _rep in range(reps):
            _build_body(nc, tc, d)
    return nc


def _build_body(nc, tc, d):
    x_d, xq_d, wT_d, projTh_d = d["x"], d["xq"], d["wT"], d["projTh"]
    pvec_d, projbr_d, G_d, GT_d, out_d = d["pvec"], d["projbr"], d["G"], d["GT"], d["out"]

    with (
        tc.tile_pool(name="singles", bufs=1) as singles,
        tc.tile_pool(name="pt", bufs=3) as ptp,
        tc.tile_pool(name="epi", bufs=2) as epi,
        tc.tile_pool(name="outp", bufs=2) as outp,
        tc.tile_pool(name="pg", bufs=4) as pg,
        tc.tile_pool(name="sps", bufs=2, space="PSUM") as sps,
        tc.tile_pool(name="accps", bufs=2, space="PSUM") as accps,
        tc.tile_pool(name="ops", bufs=1, space="PSUM") as ops,
    ):
        # ---- constants: DMA + fp16 conversion (staging pool freed after) ----
        pvec_sb = singles.tile([128, 8], F32, tag="pvec")
        nc.sync.dma_start(out=pvec_sb, in_=pvec_d[:, :])
        G_sb = singles.tile([128, 16], F32, tag="G")
        nc.sync.dma_start(out=G_sb, in_=G_d[:, :])
        GT_sb = singles.tile([16, 128], F32, tag="GT")
        nc.sync.dma_start(out=GT_sb, in_=GT_d[:, :])
        xq_sb = singles.tile([128, 2, NQC], F32, tag="xq")
        nc.sync.dma_start(out=xq_sb[:, 0, :], in_=xq_d[0:128, :])
        nc.sync.dma_start(out=xq_sb[:, 1, :], in_=xq_d[128:256, :])

        wT_h = singles.tile([128, 2, 3 * C], F16, tag="wT_h")
        projTh_h = singles.tile([64, 4, C], F16, tag="projTh_h")
        projbr_h = singles.tile([1, C], F16, tag="projbr_h")
        with tc.tile_pool(name="wstage", bufs=1) as wstage:
            # fp16 conversions on ScalarE — DVE is the serial startup chain
            # (bn_stats), ScalarE is idle here
            ws = wstage.tile([128, 2, 3 * C], F32, tag="ws")
            nc.sync.dma_start(out=ws[:, 0, :], in_=wT_d[0:128, :])
            nc.sync.dma_start(out=ws[:, 1, :], in_=wT_d[128:256, :])
            nc.scalar.activation(out=wT_h[:, 0, :], in_=ws[:, 0, :], func=AF.Copy)
            nc.scalar.activation(out=wT_h[:, 1, :], in_=ws[:, 1, :], func=AF.Copy)
            ps = wstage.tile([64, 4 * C], F32, tag="ps")
            nc.sync.dma_start(out=ps, in_=projTh_d[:, :])
            nc.scalar.activation(out=projTh_h.rearrange("p h c -> p (h c)"), in_=ps, func=AF.Copy)
            pbs = wstage.tile([1, C], F32, tag="pbs")
            nc.sync.dma_start(out=pbs, in_=projbr_d[:, :])
            nc.scalar.activation(out=projbr_h, in_=pbs, func=AF.Copy)

        ones_h = singles.tile([1, 512], F16, tag="ones_h")
        nc.vector.memset(ones_h, 1.0)
        ones_c = singles.tile([128, 1], F16, tag="ones_c")
        nc.vector.memset(ones_c, 1.0)

        # ---- persistent fp16 state ----
        xn = singles.tile([128, 2, SEQ], F16, tag="xn")          # raw x, then normalized
        k_res = singles.tile([128, 2, SEQ], F16, tag="k_res")    # [2-head rows, hp, token]
        v_res = singles.tile([128, NCH, 2, 130], F16, tag="v_res")  # [tok%128, chunk, hp, 2x(64|1)]
        q_res = singles.tile([128, 2, 3, 512], F16, tag="q_res")  # [2-head rows, hp, window, q]
        attn_sb = singles.tile([64, HEADS, NQC], F16, tag="attn")  # [dim, head, q]
        xnq = singles.tile([128, 2, NQC], F16, tag="xnq")

        # ones columns of v (col 64 of each head slot) — written once
        v5 = v_res.rearrange("p ch hp (hl c) -> p ch hp hl c", hl=2)
        ones_bc = bass.AP(tensor=ones_c.tensor, offset=ones_c.offset,
                          ap=[list(ones_c.ap[0]), [0, NCH], [0, 2], [0, 2], [1, 1]])
        nc.vector.tensor_copy(out=v5[:, :, :, :, 64:65], in_=ones_bc)

        # ---- phase 1: x (fp16, host-cast) streamed straight into xn on 3
        # DMA queues + bn_stats ----
        stats = singles.tile([128, 2, NST, 6], F32, tag="stats")
        qeng = [nc.gpsimd, nc.scalar, nc.sync]
        for bt in range(NST // 2):  # 9 big tiles of [128, 1024] per cc
            for cc in range(2):
                sl = slice(2 * ST * bt, 2 * ST * (bt + 1))
                qeng[(2 * bt + cc) % 3].dma_start(
                    out=xn[:, cc, sl], in_=x_d[128 * cc:128 * (cc + 1), sl])
                nc.vector.bn_stats(out=stats[:, cc, 2 * bt, :],
                                   in_=xn[:, cc, 2 * ST * bt:2 * ST * bt + ST])
                nc.vector.bn_stats(out=stats[:, cc, 2 * bt + 1, :],
                                   in_=xn[:, cc, 2 * ST * bt + ST:2 * ST * (bt + 1)])

        # ---- phase 2: GroupNorm stats -> per-channel a, b ----
        ab_sb = singles.tile([128, 2, 2], F32, tag="ab")  # [:, cc, 0]=a, [:, cc, 1]=b
        for cc in range(2):
            mv = pg.tile([128, 2], F32, tag="mv")
            nc.vector.bn_aggr(out=mv, in_=stats[:, cc, :, :])
            st2 = pg.tile([128, 2], F32, tag="st2")  # (mean, E[x^2])
            nc.vector.tensor_copy(out=st2[:, 0:1], in_=mv[:, 0:1])
            nc.vector.tensor_tensor(out=st2[:, 1:2], in0=mv[:, 0:1], in1=mv[:, 0:1], op=ALU.mult)
            nc.vector.tensor_tensor(out=st2[:, 1:2], in0=st2[:, 1:2], in1=mv[:, 1:2], op=ALU.add)
            gps = accps.tile([128, 512], F32, tag="acc")
            nc.tensor.matmul(gps[0:16, 0:2], lhsT=G_sb, rhs=st2, start=True, stop=True)
            gm = pg.tile([16, 2], F32, tag="gm")  # (mean_g, E2_g)
            nc.vector.tensor_copy(out=gm, in_=gps[0:16, 0:2])
            t16 = pg.tile([16, 1], F32, tag="t16")
            nc.vector.tensor_tensor(out=t16, in0=gm[:, 0:1], in1=gm[:, 0:1], op=ALU.mult)
            nc.vector.tensor_tensor(out=gm[:, 1:2], in0=gm[:, 1:2], in1=t16, op=ALU.subtract)
            # rstd = 1/sqrt(var+eps)
            nc.vector.tensor_scalar_add(out=gm[:, 1:2], in0=gm[:, 1:2], scalar1=EPS)
            nc.scalar.activation(out=gm[:, 1:2], in_=gm[:, 1:2], func=AF.Sqrt)
            nc.vector.reciprocal(out=gm[:, 1:2], in_=gm[:, 1:2])
            mps = accps.tile([128, 512], F32, tag="acc")
            nc.tensor.matmul(mps[0:128, 0:2], lhsT=GT_sb, rhs=gm, start=True, stop=True)
            mr = pg.tile([128, 2], F32, tag="mr")  # (mean_c, rstd_c)
            nc.vector.tensor_copy(out=mr, in_=mps[0:128, 0:2])
            # a = rstd * norm_w ; b = norm_b - mean * a
            nc.vector.tensor_tensor(out=ab_sb[:, cc, 0:1], in0=mr[:, 1:2], in1=pvec_sb[:, 4 + cc:5 + cc], op=ALU.mult)
            t128 = pg.tile([128, 1], F32, tag="t128")
            nc.vector.tensor_tensor(out=t128, in0=mr[:, 0:1], in1=ab_sb[:, cc, 0:1], op=ALU.mult)
            nc.vector.tensor_tensor(out=ab_sb[:, cc, 1:2], in0=pvec_sb[:, 6 + cc:7 + cc], in1=t128, op=ALU.subtract)

        # ---- phase 3: xq -> xnq (xn normalized per-tile inside the qkv loop) ----
        for cc in range(2):
            nc.vector.tensor_scalar(
                out=xnq[:, cc, :], in0=xq_sb[:, cc, :],
                scalar1=ab_sb[:, cc, 0:1], scalar2=ab_sb[:, cc, 1:2],
                op0=ALU.mult, op1=ALU.add)

        # ---- phase 4: qkv for all 4 heads ----
        def emit_qkv_tile(st):
            s0 = ST * st
            for cc in range(2):  # normalize this tile's xn slice in place (fp16 2x DVE)
                nc.vector.tensor_scalar(
                    out=xn[:, cc, s0:s0 + ST], in0=xn[:, cc, s0:s0 + ST],
                    scalar1=ab_sb[:, cc, 0:1], scalar2=ab_sb[:, cc, 1:2],
                    op0=ALU.mult, op1=ALU.add)
            for kb in range(2):  # k rows [128kb:128kb+128] = head-pair kb
                kps = accps.tile([128, 512], F32, tag="acc", name="kps")
                for cc in range(2):
                    nc.tensor.matmul(
                        kps, lhsT=wT_h[:, cc, C + 128 * kb:C + 128 * kb + 128],
                        rhs=xn[:, cc, s0:s0 + ST], start=(cc == 0), stop=(cc == 1))
                nc.vector.tensor_scalar_add(
                    out=k_res[:, kb, s0:s0 + ST], in0=kps,
                    scalar1=pvec_sb[:, 2 + kb:3 + kb])
            for mc in range(4):  # token sub-chunks of 128
                vps = accps.tile([128, 512], F32, tag="acc", name="vps")
                for cc in range(2):
                    nc.tensor.matmul(
                        vps[:, 0:256],
                        lhsT=xn[:, cc, s0 + 128 * mc:s0 + 128 * (mc + 1)],
                        rhs=wT_h[:, cc, 2 * C:3 * C],
                        start=(cc == 0), stop=(cc == 1))
                ch = 4 * st + mc
                vpsv = vps[:, 0:256].rearrange("p (hp hl c) -> p hp hl c", hp=2, hl=2)
                nc.vector.tensor_copy(out=v5[:, ch, :, :, 0:64], in_=vpsv)

        def emit_q():
            for w in range(3):
                qn = WINDOWS[w][1]
                for kb in range(2):
                    qps = accps.tile([128, 512], F32, tag="acc", name="qps")
                    for cc in range(2):
                        nc.tensor.matmul(
                            qps[:, 0:qn], lhsT=wT_h[:, cc, 128 * kb:128 * kb + 128],
                            rhs=xnq[:, cc, 512 * w:512 * w + qn], start=(cc == 0), stop=(cc == 1))
                    nc.vector.tensor_scalar_add(out=q_res[:, kb, w, 0:qn],
                                                in0=qps[:, 0:qn], scalar1=pvec_sb[:, kb:kb + 1])

        # ---- attention ----
        o_tiles = {}

        def gen_attention(hp, w):
            """Generator: one yield per S-tile group (for interleaved emission)."""
            q0, qn, kc0, nch = WINDOWS[w]
            o_t = {hl: ops.tile([128, 512], F32, tag=f"o{hl}", name=f"o{hl}") for hl in range(2)}
            o_tiles[(hp, w)] = o_t
            stream = [(hl, kc0 + c) for c in range(nch) for hl in range(2)]
            groups = [stream[i:i + SCHUNK] for i in range(0, len(stream), SCHUNK)]
            s_tiles = {}

            def emit_qk(g):
                # each QK matmul output must start on a PSUM bank boundary
                s_ps = sps.tile([128, 2, 512], F32, tag="s", name="s_ps")
                for j, (hl, kc) in enumerate(groups[g]):
                    nc.tensor.matmul(
                        s_ps[:, j, 0:qn],
                        lhsT=k_res[64 * hl:64 * hl + 64, hp, 128 * kc:128 * kc + 128],
                        rhs=q_res[64 * hl:64 * hl + 64, hp, w, 0:qn],
                        start=True, stop=True)
                s_tiles[g] = s_ps

            emit_qk(0)
            for g in range(len(groups)):
                if g + 1 < len(groups):
                    emit_qk(g + 1)
                items = groups[g]
                m = len(items)
                s_ps = s_tiles.pop(g)
                pt = ptp.tile([128, 2 * 512], F16, tag="p", name="pt")
                ptv = pt[:, 0:m * qn].rearrange("p (j c) -> p j c", j=m)
                nc.scalar.activation(out=ptv, in_=s_ps[:, 0:m, 0:qn], func=AF.Exp, scale=SCALE)
                for j, (hl, kc) in enumerate(items):
                    nc.tensor.matmul(
                        o_t[hl][0:65, 0:qn],
                        lhsT=v5[:, kc, hp, hl, :],
                        rhs=pt[:, qn * j:qn * (j + 1)],
                        start=(kc == kc0), stop=(kc == kc0 + nch - 1))
                yield

        def drive(gen, inject=None):
            """Consume gen; inject[-1] fns emit before it starts, inject[g]
            right after group g (epilogues/projections of earlier windows, so
            they overlap this window's attention)."""
            inject = inject or {}
            for fn in inject.get(-1, []):
                fn()
            g = 0
            for _ in gen:
                for fn in inject.get(g, []):
                    fn()
                g += 1

        def make_epilogue(hp, w):
            """O^T[0:64] / O^T[64] -> attn[0:64, head, q]  (all on partitions 0:64)."""
            q0, qn, _, _ = WINDOWS[w]
            o_t = o_tiles[(hp, w)]

            def fn():
                for hl in range(2):
                    osb = epi.tile([65, 512], F32, tag="osb", name="osb")
                    nc.vector.tensor_copy(out=osb[:, 0:qn], in_=o_t[hl][0:65, 0:qn])
                    # sums row (f16, base partition 0) -> broadcast [64, qn]
                    # on PE (rank-1 with ones), then reciprocal+mult on DVE
                    # across all 64 lanes at once
                    srow = epi.tile([1, 512], F16, tag="srow", name="srow")
                    nc.vector.tensor_copy(out=srow[:, 0:qn], in_=osb[64:65, 0:qn])
                    bps = accps.tile([128, 512], F32, tag="acc", name="bps")
                    nc.tensor.matmul(bps[0:64, 0:qn], lhsT=ones_h[0:1, 0:64],
                                     rhs=srow[:, 0:qn], start=True, stop=True)
                    recb = epi.tile([64, 512], F32, tag="recb", name="recb")
                    nc.vector.reciprocal(out=recb[:, 0:qn], in_=bps[0:64, 0:qn])
                    nc.vector.tensor_tensor(
                        out=attn_sb[:, 2 * hp + hl, q0:q0 + qn],
                        in0=osb[0:64, 0:qn], in1=recb[:, 0:qn], op=ALU.mult)
            return fn

        def make_proj(w):
            q0, qn, _, _ = WINDOWS[w]

            def fn():
                for mc in range(2):
                    pp = accps.tile([128, 512], F32, tag="acc", name="pp")
                    nc.tensor.matmul(pp[:, 0:qn], lhsT=projbr_h[0:1, 128 * mc:128 * (mc + 1)],
                                     rhs=ones_h[0:1, 0:qn], start=True, stop=False)
                    for h in range(HEADS):
                        nc.tensor.matmul(pp[:, 0:qn], lhsT=projTh_h[:, h, 128 * mc:128 * (mc + 1)],
                                         rhs=attn_sb[:, h, q0:q0 + qn],
                                         start=False, stop=(h == HEADS - 1))
                    ot = outp.tile([128, 512], F32, tag="ot")
                    nc.vector.tensor_tensor(out=ot[:, 0:qn], in0=pp[:, 0:qn],
                                            in1=xq_sb[:, mc, q0:q0 + qn], op=ALU.add)
                    nc.sync.dma_start(out=out_d[128 * mc:128 * (mc + 1), q0:q0 + qn], in_=ot[:, 0:qn])
            return fn

        # qkv emission interleaved with sweep-A W0 attention: pump a W0 group
        # as soon as the key-chunks its (one-ahead) QK reads are written.
        emit_q()
        gA0 = gen_attention(0, 0)
        NG0 = (2 * WINDOWS[0][3] + SCHUNK - 1) // SCHUNK
        pumped = 0
        for st in range(NST):
            emit_qkv_tile(st)
            while pumped < NG0 and (SCHUNK * (pumped + 2) - 1) // 2 <= 4 * st + 3:
                next(gA0, None)
                pumped += 1
        for _ in gA0:
            pass
        # remaining sweep A, then sweep B; window w-1's epilogue is injected
        # before window w's first PV (o_t bank reuse needs it emitted first),
        # projections injected a few groups in.
        drive(gen_attention(0, 1), {-1: [make_epilogue(0, 0)]})
        drive(gen_attention(0, 2), {-1: [make_epilogue(0, 1)]})
        drive(gen_attention(1, 0), {-1: [make_epilogue(0, 2)]})
        drive(gen_attention(1, 1), {-1: [make_epilogue(1, 0)], 3: [make_proj(0)]})
        drive(gen_attention(1, 2), {-1: [make_epilogue(1, 1)], 3: [make_proj(1)]})
        make_epilogue(1, 2)()
        make_proj(2)()


def make_inputs(x, norm_w, norm_b, qkv_w, qkv_b, proj_w, proj_b):
    """Host-side prep: full-input numpy -> per-core in_maps."""
    x2 = np.ascontiguousarray(np.asarray(x, np.float32).reshape(C, SEQ))
    x16 = x2.astype(np.float16)
    qkv_w = np.asarray(qkv_w, np.float32)
    qkv_b = np.asarray(qkv_b, np.float32)
    proj_w = np.asarray(proj_w, np.float32)
    proj_b = np.asarray(proj_b, np.float32)
    norm_w = np.asarray(norm_w, np.float32)
    norm_b = np.asarray(norm_b, np.float32)

    wT = np.ascontiguousarray(qkv_w.T)
    projT = np.ascontiguousarray(proj_w.T)  # [c_in, c_out]
    projTh = np.ascontiguousarray(
        projT.reshape(4, 64, C).transpose(1, 0, 2).reshape(64, 4 * C))
    # v-bias folds into the projection bias: proj(attn + bv) = proj(attn) + proj_w @ bv
    projbr = (proj_b + proj_w @ qkv_b[2 * C:3 * C]).reshape(1, C).astype(np.float32)
    pvec = np.stack([
        qkv_b[0:128], qkv_b[128:256],            # q bias bank 0/1
        qkv_b[C:C + 128], qkv_b[C + 128:2 * C],  # k bias bank 0/1
        norm_w[0:128], norm_w[128:256],
        norm_b[0:128], norm_b[128:256],
    ], axis=1).astype(np.float32)
    cidx = np.arange(128)
    gidx = np.arange(16)
    G = ((cidx[:, None] // 8) == gidx[None, :]).astype(np.float32) / 8.0
    GT = np.ascontiguousarray(G.T * 8.0)

    common = dict(x=x16, wT=wT, projTh=projTh, pvec=pvec, projbr=projbr, G=G, GT=GT)
    in_maps = []
    cols = []
    for i in range(NCORES):
        ci = np.concatenate([
            np.arange(512 * i, 512 * (i + 1)),
            np.arange(4096 + 512 * i, 4096 + 512 * (i + 1)),
            np.arange(8192 + 128 * i, 8192 + 128 * (i + 1)),
        ])
        cols.append(ci)
        m = dict(common)
        m["xq"] = np.ascontiguousarray(x2[:, ci])
        in_maps.append(m)
    return in_maps, cols


_NC_CACHE = {}


def kernel(x, norm_w, norm_b, qkv_w, qkv_b, proj_w, proj_b):
    from concourse.bass_utils import run_bass_kernel_spmd

    _patch_tile_drain()
    _patch_to_json_split_waits()
    in_maps, cols = make_inputs(x, norm_w, norm_b, qkv_w, qkv_b, proj_w, proj_b)
    if "nc" not in _NC_CACHE:
        _NC_CACHE["nc"] = build_nc()
    nc = _NC_CACHE["nc"]
    res = run_bass_kernel_spmd(nc, in_maps, core_ids=list(range(NCORES)))
    out = np.zeros((C, SEQ), np.float32)
    for i in range(NCORES):
        out[:, cols[i]] = res.results[i]["out"]
    return out.reshape(1, C, 96, 96)
